# revision 9
# baseline (speedup 1.0000x reference)
"""Bayesian linear layer (sample branch) on 8 Trainium2 NeuronCores.

Sharding: 8-way over the OUT dimension (128 output features per core).
The 256MB eps_w tensor is pre-cast to fp16 and pre-tiled on the host, so
each core streams a contiguous 16MB tile sequence at full DMA rate:

  eps_t[c, g, p, (bl o)] = fp16(eps_w[g*BG+bl, o_shard+o, 128c+p])

Per tile [128 i-part, (BG b x 128 o) free]:
    DVE : s   = sigmaT (x) eps16              (fp16 TT, 2x mode)
          w2, Sw2 = SQADD custom op: sq(s + muT) with fused accumulate
          (on ~40% of tiles) q = (1+E) pairwise products (PAIR custom op)
    ACT : E   = exp(-c2*w2 + c1)              (one table set: exp+ln)
          SL += ln(E + 1)  or  ln(q) on paired tiles
    PE  : out[o,b] += s_tile^T @ x_col        (matvec, PSUM accum)
          Se2 via eps-Gram diag: psum += e_tile^T @ e_tile
plus a dense fp32 mu-matmul on PE and a tiny bias pipeline.  Scalar
partial sums leave as per-partition vectors, reduced on the host.

Exact identities:
  log_prior elem = a + softplus(d),  a = log(.5)-LOG_SQRT_2PI - w^2/2,
                   d = -log(.002) - (125000-0.5) w^2
  ln(1+E1) + ln(1+E2) = ln((1+E1)(1+E2))
  log_posterior  = -N*LOG_SQRT_2PI - B*Sum log sigma - Sum eps^2 / 2
"""

import os
import numpy as np

B, IN, OUT = 64, 1024, 1024
NCORES = 8
O_LOC = OUT // NCORES      # 128
P = 128
NCH = IN // P              # 8 i-chunks
BG = 32                    # batches per tile
NG = B // BG               # 2 tile groups over batch
NTILES = NCH * NG          # 16
PAIR_EVERY = 6             # tiles with (t % PAIR_EVERY) < PAIR_CNT use pairing
PAIR_CNT = 1

LOG_SQRT_2PI = 0.9189385332046727
C1 = 6.214608098422191     # -log(0.002)
C2 = 124999.5              # 1/(2*0.002^2) - 1/2
A_CONST = float(np.log(0.5) - LOG_SQRT_2PI)

_CACHE = {}
LAST_EXEC_NS = None


def _patch_act_tables():
    """Force every activation onto the one table set that holds both exp
    and ln, so the kernel does a single ACT_TABLE_LOAD instead of
    thrashing between exp_and_others and natural_log per instruction."""
    import concourse.bacc as bacc_mod

    if getattr(bacc_mod, "_ant_single_act_set", False):
        return
    orig = bacc_mod.get_activation_tables

    def patched(arch):
        t = orig(arch)
        return {
            name: (fns if name == "natural_log_exp_and_others" else set())
            for name, fns in t.items()
        }

    bacc_mod.get_activation_tables = patched
    bacc_mod._ant_single_act_set = True


def _register_custom_ops():
    """Register two fused DVE micro-op programs:
       SQADD_REDUCE_ANT: out = (in0 + in1)^2 ; accum_out = sum(out)
       PAIR1P_ANT:       out = (in0 + s0) * (in1 + s0)
    Appended to dve_ops.OPS at runtime; shas computed on the fly."""
    from concourse import dve_ops as dops
    from concourse.dve_spec import Spec, Src0, Src1, Zero, C0, sq, lower
    from concourse.dve_spec import _has_src1 as has_src1
    from concourse.dve_uop import DveOpSpec
    from operator import add as _add

    if "SQADD_REDUCE_ANT" in dops._SUB_OPCODE_FOR_NAME:
        by_name = {op.name: op for op in dops.OPS}
        return by_name["SQADD_REDUCE_ANT"], by_name["PAIR1P_ANT"]

    def _ref_sqadd(in0, in1, c0, c1, c2):
        b = ((in0.astype(np.float32) + in1) ** 2).astype(np.float32)
        return b, b.reshape(b.shape[0], -1).sum(axis=-1, keepdims=True)

    def _ref_pair(in0, in1, s0, s1, imm2):
        return (in0.astype(np.float32) + s0) * (in1.astype(np.float32) + s0)

    def _register(name, spec, subdim=False):
        row = max(dops._SUB_OPCODE_FOR_NAME.values()) + 1
        assert row < 0x20
        dops._SUB_OPCODE_FOR_NAME[name] = row
        shas = {}
        for ver in ("v3", "v4"):
            s = DveOpSpec(name=name, opcode=row, uops=lower(spec, ver=ver),
                          rd1_en=has_src1(spec))
            shas[ver] = s.sha(ver)
        op = dops.DveOp(name, spec, subdim=subdim, uops_sha=shas)
        dops.OPS.append(op)
        dops.CUSTOM_DVE_SPECS[name] = spec
        return op

    sqadd = _register(
        "SQADD_REDUCE_ANT",
        Spec(body=sq(Src0 + Src1), accum=_add, accum_init=Zero,
             reference=_ref_sqadd))
    pair = _register(
        "PAIR1P_ANT",
        Spec(body=(Src0 + C0) * (Src1 + C0), reference=_ref_pair))
    return sqadd, pair


def _build():
    if "nc" in _CACHE:
        return _CACHE["nc"]
    _patch_act_tables()
    OP_SQADD, OP_PAIR = _register_custom_ops()
    import concourse.mybir as mybir
    from concourse import bacc
    from concourse.tile import TileContext
    from concourse.masks import make_identity

    f32, f16 = mybir.dt.float32, mybir.dt.float16
    bf16 = mybir.dt.bfloat16
    mult, add = mybir.AluOpType.mult, mybir.AluOpType.add
    Exp, Ln = mybir.ActivationFunctionType.Exp, mybir.ActivationFunctionType.Ln

    nc = bacc.Bacc("TRN2", target_bir_lowering=False, debug=False,
                   num_devices=NCORES)

    eps_d = nc.declare_dram_parameter("eps_t", [NCH, NG, P, BG * O_LOC], f16, isOutput=False)
    mu_d = nc.declare_dram_parameter("mu_t", [NCH, P, O_LOC], f32, isOutput=False)
    rho_d = nc.declare_dram_parameter("rho_t", [NCH, P, O_LOC], f32, isOutput=False)
    x_d = nc.declare_dram_parameter("x_t", [NCH, P, B], f32, isOutput=False)
    epsb_d = nc.declare_dram_parameter("eps_b_t", [O_LOC, B], f32, isOutput=False)
    bmu_d = nc.declare_dram_parameter("b_mu", [O_LOC, 1], f32, isOutput=False)
    brho_d = nc.declare_dram_parameter("b_rho", [O_LOC, 1], f32, isOutput=False)

    out_d = nc.declare_dram_parameter("out_t", [O_LOC, B], f32, isOutput=True)
    NPAR = 2 * NTILES + 1 + 1 + 4
    par_d = nc.declare_dram_parameter("partials", [P, NPAR], f32, isOutput=True)

    with TileContext(nc) as tc:
        with tc.tile_pool(name="persist", bufs=1) as pp, \
             tc.tile_pool(name="big", bufs=4) as bigp, \
             tc.tile_pool(name="work", bufs=3) as wp, \
             tc.tile_pool(name="small", bufs=2) as smp, \
             tc.tile_pool(name="psum", bufs=1, space="PSUM") as psp:

            c1_ap = pp.tile([P, 1], f32, tag="c1")
            nc.vector.memset(c1_ap[:], C1)
            ident = pp.tile([P, P], f32, tag="ident")
            make_identity(nc, ident[:])

            acc_w2 = pp.tile([P, NTILES], f32, tag="acc_w2")
            acc_L = pp.tile([P, NTILES], f32, tag="acc_L")
            acc_e2 = pp.tile([P, 1], f32, tag="acc_e2")
            acc_ls = pp.tile([P, 1], f32, tag="acc_ls")
            acc_bias = pp.tile([P, 4], f32, tag="acc_bias")

            mv_ps = psp.tile([O_LOC, B], f32, tag="mv")
            gram_ps = psp.tile([P, P], f32, tag="gram")

            # ---- preamble: one wide [128, NCH*...] chain ----
            mu_all = pp.tile([P, NCH * O_LOC], f32, tag="mu_all")
            nc.sync.dma_start(
                out=mu_all[:].rearrange("p (c o) -> p c o", c=NCH),
                in_=mu_d[:].rearrange("c p o -> p c o"))
            x_all = pp.tile([P, NCH * B], f32, tag="x_all")
            nc.sync.dma_start(
                out=x_all[:].rearrange("p (c b) -> p c b", c=NCH),
                in_=x_d[:].rearrange("c p b -> p c b"))
            rho_all = pp.tile([P, NCH * O_LOC], f32, tag="rho_all")
            nc.sync.dma_start(
                out=rho_all[:].rearrange("p (c o) -> p c o", c=NCH),
                in_=rho_d[:].rearrange("c p o -> p c o"))
            er_all = pp.tile([P, NCH * O_LOC], f32, tag="er_all")
            nc.scalar.activation(out=er_all[:], in_=rho_all[:], func=Exp)
            sig32_all = pp.tile([P, NCH * O_LOC], f32, tag="sig32_all")
            nc.scalar.activation(out=sig32_all[:], in_=er_all[:], func=Ln, bias=1.0)
            lnscr = pp.tile([P, NCH * O_LOC], f32, tag="lnscr")
            nc.scalar.activation(out=lnscr[:], in_=sig32_all[:], func=Ln,
                                 accum_out=acc_ls[:, 0:1])
            sig16_all = pp.tile([P, NCH * O_LOC], f16, tag="sig16_all")
            nc.vector.tensor_copy(out=sig16_all[:], in_=sig32_all[:])
            mu16_all = pp.tile([P, NCH * O_LOC], f16, tag="mu16_all")
            nc.vector.tensor_copy(out=mu16_all[:], in_=mu_all[:])
            x16_all = pp.tile([P, NCH * B], f16, tag="x16_all")
            nc.vector.tensor_copy(out=x16_all[:], in_=x_all[:])
            sig16 = [sig16_all[:, c * O_LOC:(c + 1) * O_LOC] for c in range(NCH)]
            mu16 = [mu16_all[:, c * O_LOC:(c + 1) * O_LOC] for c in range(NCH)]
            x16 = [x16_all[:, c * B:(c + 1) * B] for c in range(NCH)]
            mu32 = [mu_all[:, c * O_LOC:(c + 1) * O_LOC] for c in range(NCH)]
            x32 = [x_all[:, c * B:(c + 1) * B] for c in range(NCH)]

            # ---- bias pipeline ----
            bmu = pp.tile([P, 1], f32, tag="bmu")
            nc.sync.dma_start(out=bmu[:], in_=bmu_d[:])
            brho = pp.tile([P, 1], f32, tag="brho")
            nc.sync.dma_start(out=brho[:], in_=brho_d[:])
            eb = pp.tile([P, B], f32, tag="eb")
            nc.sync.dma_start(out=eb[:], in_=epsb_d[:])

            erb = wp.tile([P, 1], f32, tag="erb")
            nc.scalar.activation(out=erb[:], in_=brho[:], func=Exp)
            bsig = pp.tile([P, 1], f32, tag="bsig")
            nc.scalar.activation(out=bsig[:], in_=erb[:], func=Ln, bias=1.0)
            lnb = wp.tile([P, 1], f32, tag="lnb")
            nc.scalar.activation(out=lnb[:], in_=bsig[:], func=Ln,
                                 accum_out=acc_bias[:, 3:4])
            wb = pp.tile([P, B], f32, tag="wb")
            nc.vector.tensor_scalar(out=wb[:], in0=eb[:], scalar1=bsig[:, 0:1],
                                    scalar2=bmu[:, 0:1], op0=mult, op1=add)
            wb2 = wp.tile([P, B], f32, tag="wb2")
            nc.vector.scalar_tensor_tensor(out=wb2[:], in0=wb[:], scalar=1.0,
                                           in1=wb[:], op0=mult, op1=mult,
                                           accum_out=acc_bias[:, 0:1])
            Eb = wp.tile([P, B], f32, tag="Eb")
            nc.scalar.activation(out=Eb[:], in_=wb2[:], func=Exp,
                                 scale=-C2, bias=c1_ap[:, 0:1])
            Lb = wp.tile([P, B], f32, tag="Lb")
            nc.scalar.activation(out=Lb[:], in_=Eb[:], func=Ln, bias=1.0,
                                 accum_out=acc_bias[:, 1:2])
            eb2 = wp.tile([P, B], f32, tag="eb2")
            nc.vector.scalar_tensor_tensor(out=eb2[:], in0=eb[:], scalar=1.0,
                                           in1=eb[:], op0=mult, op1=mult,
                                           accum_out=acc_bias[:, 2:3])

            # ---- dense mu matmul: out[o,b] = sum_i mu[o,i] x[b,i] (fp32) ----
            for c in range(NCH):
                nc.tensor.matmul(out=mv_ps[:], lhsT=mu32[c], rhs=x32[c],
                                 start=(c == 0), stop=False)

            # ---- main eps stream ----
            for t in range(NTILES):
                c, g = divmod(t, NG)
                use_pair = (t % PAIR_EVERY) < PAIR_CNT
                e16 = bigp.tile([P, BG * O_LOC], f16, tag="e16")
                nc.sync.dma_start(out=e16[:], in_=eps_d[c, g])

                e16v = e16[:].rearrange("p (b o) -> p b o", b=BG)
                sig_bc = sig16[c].unsqueeze(1).broadcast_to([P, BG, O_LOC])
                mu_bc = mu16[c].unsqueeze(1).broadcast_to([P, BG, O_LOC])

                s16 = bigp.tile([P, BG * O_LOC], f16, tag="s16")
                s16v = s16[:].rearrange("p (b o) -> p b o", b=BG)
                nc.vector.tensor_tensor(out=s16v, in0=e16v, in1=sig_bc, op=mult)

                # w2 = (s + mu)^2 with fused sum -> acc_w2[:, t]
                w2 = wp.tile([P, BG * O_LOC], f16, tag="w2")
                w2v = w2[:].rearrange("p (b o) -> p b o", b=BG)
                nc.vector._custom_dve(OP_SQADD, out=w2v, in0=s16v, in1=mu_bc,
                                      accum_out=acc_w2[:, t:t + 1])

                E = wp.tile([P, BG * O_LOC], f16, tag="E")
                nc.scalar.activation(out=E[:], in_=w2[:], func=Exp,
                                     scale=-C2, bias=c1_ap[:, 0:1])
                if use_pair:
                    Ev = E[:].rearrange("p (n two) -> p n two", two=2)
                    q = smp.tile([P, BG * O_LOC // 2], bf16, tag="q")
                    nc.vector._custom_dve(OP_PAIR, out=q[:], in0=Ev[:, :, 0],
                                          in1=Ev[:, :, 1], s0=1.0)
                    L = smp.tile([P, BG * O_LOC // 2], f16, tag="Lp")
                    nc.scalar.activation(out=L[:], in_=q[:], func=Ln,
                                         accum_out=acc_L[:, t:t + 1])
                else:
                    L = smp.tile([P, BG * O_LOC], f16, tag="L")
                    nc.scalar.activation(out=L[:], in_=E[:], func=Ln, bias=1.0,
                                         accum_out=acc_L[:, t:t + 1])

                for bl in range(BG):
                    b = g * BG + bl
                    last = (t == NTILES - 1 and bl == BG - 1)
                    nc.tensor.matmul(out=mv_ps[:, b:b + 1],
                                     lhsT=s16v[:, bl, :],
                                     rhs=x16[c][:, b:b + 1],
                                     start=False, stop=last)
                    nc.tensor.matmul(out=gram_ps[:],
                                     lhsT=e16v[:, bl, :],
                                     rhs=e16v[:, bl, :],
                                     start=(t == 0 and bl == 0), stop=last)

            # ---- finalize ----
            gmask = wp.tile([P, P], f32, tag="gmask")
            nc.vector.tensor_tensor(out=gmask[:], in0=gram_ps[:], in1=ident[:],
                                    op=mult)
            nc.vector.tensor_reduce(out=acc_e2[:], in_=gmask[:],
                                    axis=mybir.AxisListType.X,
                                    op=add)

            out_sb = wp.tile([O_LOC, B], f32, tag="out_sb")
            nc.vector.tensor_tensor(out=out_sb[:], in0=mv_ps[:], in1=wb[:], op=add)
            nc.sync.dma_start(out=out_d[:], in_=out_sb[:])

            pk = smp.tile([P, 2 * NTILES + 1 + 1 + 4], f32, tag="pk")
            nc.vector.tensor_copy(out=pk[:, 0:NTILES], in_=acc_w2[:])
            nc.vector.tensor_copy(out=pk[:, NTILES:2 * NTILES], in_=acc_L[:])
            nc.vector.tensor_copy(out=pk[:, 2 * NTILES:2 * NTILES + 1], in_=acc_e2[:])
            nc.vector.tensor_copy(out=pk[:, 2 * NTILES + 1:2 * NTILES + 2], in_=acc_ls[:])
            nc.vector.tensor_copy(out=pk[:, 2 * NTILES + 2:], in_=acc_bias[:])
            nc.sync.dma_start(out=par_d[:], in_=pk[:])

    nc.compile()
    _CACHE["nc"] = nc
    return nc


def _prepare_in_maps(x, weight_mu, weight_rho, bias_mu, bias_rho, eps_w, eps_b):
    x = np.asarray(x, np.float32)
    weight_mu = np.asarray(weight_mu, np.float32)
    weight_rho = np.asarray(weight_rho, np.float32)
    bias_mu = np.asarray(bias_mu, np.float32)
    bias_rho = np.asarray(bias_rho, np.float32)
    eps_w = np.asarray(eps_w, np.float32)
    eps_b = np.asarray(eps_b, np.float32)

    # x_t[c, p, b] = x[b, 128c+p]  (shared by all cores)
    x_t = np.ascontiguousarray(x.reshape(B, NCH, P).transpose(1, 2, 0))
    eps16 = eps_w.astype(np.float16)

    in_maps = []
    for r in range(NCORES):
        osh = slice(r * O_LOC, (r + 1) * O_LOC)
        # eps_t[c, g, p, bl*O_LOC + o] = eps16[g*BG+bl, osh.start+o, 128c+p]
        shard = eps16[:, osh, :]                       # (B, O_LOC, IN)
        eps_t = np.ascontiguousarray(
            shard.reshape(NG, BG, O_LOC, NCH, P).transpose(3, 0, 4, 1, 2)
            .reshape(NCH, NG, P, BG * O_LOC))
        mu_t = np.ascontiguousarray(weight_mu[osh].T.reshape(NCH, P, O_LOC))
        rho_t = np.ascontiguousarray(weight_rho[osh].T.reshape(NCH, P, O_LOC))
        in_maps.append({
            "eps_t": eps_t,
            "mu_t": mu_t,
            "rho_t": rho_t,
            "x_t": x_t,
            "eps_b_t": np.ascontiguousarray(eps_b[:, osh].T),
            "b_mu": np.ascontiguousarray(bias_mu[osh].reshape(O_LOC, 1)),
            "b_rho": np.ascontiguousarray(bias_rho[osh].reshape(O_LOC, 1)),
        })
    return in_maps


def kernel(x, weight_mu, weight_rho, bias_mu, bias_rho, eps_w, eps_b):
    global LAST_EXEC_NS
    from concourse.bass_utils import run_bass_kernel_spmd

    nc = _build()
    in_maps = _prepare_in_maps(x, weight_mu, weight_rho, bias_mu, bias_rho,
                               eps_w, eps_b)
    trace = os.environ.get("BL_TRACE", "0") == "1"
    kw = {}
    td = os.environ.get("BL_TMPDIR")
    if td:
        os.makedirs(td, exist_ok=True)
        kw["tmpdir"] = td
    res = run_bass_kernel_spmd(nc, in_maps, list(range(NCORES)), trace=trace, **kw)
    LAST_EXEC_NS = res.exec_time_ns
    _CACHE["last_results"] = res

    out = np.concatenate([res.results[r]["out_t"].T for r in range(NCORES)],
                         axis=1).astype(np.float32)

    sw2 = sL = se2 = sls = 0.0
    swb2 = sLb = seb2 = slbs = 0.0
    for r in range(NCORES):
        p = res.results[r]["partials"].astype(np.float64)
        sw2 += p[:, 0:NTILES].sum()
        sL += p[:, NTILES:2 * NTILES].sum()
        se2 += p[:, 2 * NTILES].sum()
        sls += p[:, 2 * NTILES + 1].sum()
        swb2 += p[:, 2 * NTILES + 2].sum()
        sLb += p[:, 2 * NTILES + 3].sum()
        seb2 += p[:, 2 * NTILES + 4].sum()
        slbs += p[:, 2 * NTILES + 5].sum()

    n_w = float(B) * OUT * IN
    n_b = float(B) * OUT
    log_prior = (A_CONST * (n_w + n_b) - 0.5 * (sw2 + swb2) + (sL + sLb))
    log_posterior = (-LOG_SQRT_2PI * (n_w + n_b)
                     - B * (sls + slbs) - 0.5 * (se2 + seb2))
    return out, np.float32(log_prior), np.float32(log_posterior)


# revision 11
# speedup vs baseline: 1.0801x; 1.0801x over previous
"""Bayesian linear layer (sample branch) on 8 Trainium2 NeuronCores.

Sharding: 8-way over the OUT dimension (128 output features per core).
The 256MB eps_w tensor is pre-cast to fp16 and pre-tiled on the host, so
each core streams a contiguous 16MB tile sequence at full DMA rate:

  eps_t[c, g, p, (bl o)] = fp16(eps_w[g*BG+bl, o_shard+o, 128c+p])

Per tile [128 i-part, (BG b x 128 o) free]:
    DVE : s   = sigmaT (x) eps16              (fp16 TT, 2x mode)
          w2, Sw2 = SQADD custom op: sq(s + muT) with fused accumulate
          (on ~40% of tiles) q = (1+E) pairwise products (PAIR custom op)
    ACT : E   = exp(-c2*w2 + c1)              (one table set: exp+ln)
          SL += ln(E + 1)  or  ln(q) on paired tiles
    PE  : out[o,b] += s_tile^T @ x_col        (matvec, PSUM accum)
          Se2 via eps-Gram diag: psum += e_tile^T @ e_tile
plus a dense fp32 mu-matmul on PE and a tiny bias pipeline.  Scalar
partial sums leave as per-partition vectors, reduced on the host.

Exact identities:
  log_prior elem = a + softplus(d),  a = log(.5)-LOG_SQRT_2PI - w^2/2,
                   d = -log(.002) - (125000-0.5) w^2
  ln(1+E1) + ln(1+E2) = ln((1+E1)(1+E2))
  log_posterior  = -N*LOG_SQRT_2PI - B*Sum log sigma - Sum eps^2 / 2
"""

import os
import numpy as np

B, IN, OUT = 64, 1024, 1024
NCORES = 8
O_LOC = OUT // NCORES      # 128
P = 128
NCH = IN // P              # 8 i-chunks
BG = 32                    # batches per tile
NG = B // BG               # 2 tile groups over batch
NTILES = NCH * NG          # 16
PAIR_EVERY = 3             # tiles with (t % PAIR_EVERY) < PAIR_CNT use pairing
PAIR_CNT = 1

LOG_SQRT_2PI = 0.9189385332046727
C1 = 6.214608098422191     # -log(0.002)
C2 = 124999.5              # 1/(2*0.002^2) - 1/2
A_CONST = float(np.log(0.5) - LOG_SQRT_2PI)

_CACHE = {}
LAST_EXEC_NS = None


def _patch_act_tables():
    """Force every activation onto the one table set that holds both exp
    and ln, so the kernel does a single ACT_TABLE_LOAD instead of
    thrashing between exp_and_others and natural_log per instruction."""
    import concourse.bacc as bacc_mod

    if getattr(bacc_mod, "_ant_single_act_set", False):
        return
    orig = bacc_mod.get_activation_tables

    def patched(arch):
        t = orig(arch)
        return {
            name: (fns if name == "natural_log_exp_and_others" else set())
            for name, fns in t.items()
        }

    bacc_mod.get_activation_tables = patched
    bacc_mod._ant_single_act_set = True


def _register_custom_ops():
    """Register two fused DVE micro-op programs:
       SQADD_REDUCE_ANT: out = (in0 + in1)^2 ; accum_out = sum(out)
       PAIR1P_ANT:       out = (in0 + s0) * (in1 + s0)
    Appended to dve_ops.OPS at runtime; shas computed on the fly."""
    from concourse import dve_ops as dops
    from concourse.dve_spec import Spec, Src0, Src1, Zero, C0, sq, lower
    from concourse.dve_spec import _has_src1 as has_src1
    from concourse.dve_uop import DveOpSpec
    from operator import add as _add

    if "SQADD_REDUCE_ANT" in dops._SUB_OPCODE_FOR_NAME:
        by_name = {op.name: op for op in dops.OPS}
        return by_name["SQADD_REDUCE_ANT"], by_name["PAIR1P_ANT"]

    def _ref_sqadd(in0, in1, c0, c1, c2):
        b = ((in0.astype(np.float32) + in1) ** 2).astype(np.float32)
        return b, b.reshape(b.shape[0], -1).sum(axis=-1, keepdims=True)

    def _ref_pair(in0, in1, s0, s1, imm2):
        return (in0.astype(np.float32) + s0) * (in1.astype(np.float32) + s0)

    def _register(name, spec, subdim=False):
        row = max(dops._SUB_OPCODE_FOR_NAME.values()) + 1
        assert row < 0x20
        dops._SUB_OPCODE_FOR_NAME[name] = row
        shas = {}
        for ver in ("v3", "v4"):
            s = DveOpSpec(name=name, opcode=row, uops=lower(spec, ver=ver),
                          rd1_en=has_src1(spec))
            shas[ver] = s.sha(ver)
        op = dops.DveOp(name, spec, subdim=subdim, uops_sha=shas)
        dops.OPS.append(op)
        dops.CUSTOM_DVE_SPECS[name] = spec
        return op

    sqadd = _register(
        "SQADD_REDUCE_ANT",
        Spec(body=sq(Src0 + Src1), accum=_add, accum_init=Zero,
             reference=_ref_sqadd))
    pair = _register(
        "PAIR1P_ANT",
        Spec(body=(Src0 + C0) * (Src1 + C0), reference=_ref_pair))
    return sqadd, pair


def _build():
    if "nc" in _CACHE:
        return _CACHE["nc"]
    _patch_act_tables()
    OP_SQADD, OP_PAIR = _register_custom_ops()
    import concourse.mybir as mybir
    from concourse import bacc
    from concourse.tile import TileContext
    from concourse.masks import make_identity

    f32, f16 = mybir.dt.float32, mybir.dt.float16
    bf16 = mybir.dt.bfloat16
    mult, add = mybir.AluOpType.mult, mybir.AluOpType.add
    Exp, Ln = mybir.ActivationFunctionType.Exp, mybir.ActivationFunctionType.Ln

    nc = bacc.Bacc("TRN2", target_bir_lowering=False, debug=False,
                   num_devices=NCORES)

    eps_d = nc.declare_dram_parameter("eps_t", [NCH, NG, P, BG * O_LOC], f16, isOutput=False)
    mu_d = nc.declare_dram_parameter("mu_t", [NCH, P, O_LOC], f32, isOutput=False)
    rho_d = nc.declare_dram_parameter("rho_t", [NCH, P, O_LOC], f32, isOutput=False)
    x_d = nc.declare_dram_parameter("x_t", [NCH, P, B], f32, isOutput=False)
    epsb_d = nc.declare_dram_parameter("eps_b_t", [O_LOC, B], f32, isOutput=False)
    bmu_d = nc.declare_dram_parameter("b_mu", [O_LOC, 1], f32, isOutput=False)
    brho_d = nc.declare_dram_parameter("b_rho", [O_LOC, 1], f32, isOutput=False)

    out_d = nc.declare_dram_parameter("out_t", [O_LOC, B], f32, isOutput=True)
    NPAR = 2 * NTILES + 1 + NCH + 4
    par_d = nc.declare_dram_parameter("partials", [P, NPAR], f32, isOutput=True)

    with TileContext(nc) as tc:
        with tc.tile_pool(name="persist", bufs=1) as pp, \
             tc.tile_pool(name="big", bufs=4) as bigp, \
             tc.tile_pool(name="work", bufs=3) as wp, \
             tc.tile_pool(name="small", bufs=2) as smp, \
             tc.tile_pool(name="psum", bufs=1, space="PSUM") as psp:

            c1_ap = pp.tile([P, 1], f32, tag="c1")
            nc.vector.memset(c1_ap[:], C1)
            ident = pp.tile([P, P], f32, tag="ident")
            make_identity(nc, ident[:])

            acc_w2 = pp.tile([P, NTILES], f32, tag="acc_w2")
            acc_L = pp.tile([P, NTILES], f32, tag="acc_L")
            acc_e2 = pp.tile([P, 1], f32, tag="acc_e2")
            acc_ls = pp.tile([P, NCH], f32, tag="acc_ls")
            acc_bias = pp.tile([P, 4], f32, tag="acc_bias")

            mv_ps = psp.tile([O_LOC, B], f32, tag="mv")
            gram_ps = psp.tile([P, P], f32, tag="gram")

            # ---- preamble: sigma/mu/x per i-chunk ----
            sig16 = []
            mu16 = []
            x16 = []
            mu32 = []
            x32 = []
            for c in range(NCH):
                m32 = pp.tile([P, O_LOC], f32, tag=f"mu32_{c}")
                nc.sync.dma_start(out=m32[:], in_=mu_d[c])
                mu32.append(m32)
                xx32 = pp.tile([P, B], f32, tag=f"x32_{c}")
                nc.sync.dma_start(out=xx32[:], in_=x_d[c])
                x32.append(xx32)

                rho = wp.tile([P, O_LOC], f32, tag="rho")
                nc.sync.dma_start(out=rho[:], in_=rho_d[c])
                er = wp.tile([P, O_LOC], f32, tag="er")
                nc.scalar.activation(out=er[:], in_=rho[:], func=Exp)
                s32 = wp.tile([P, O_LOC], f32, tag="s32")
                nc.scalar.activation(out=s32[:], in_=er[:], func=Ln, bias=1.0)
                lnscratch = wp.tile([P, O_LOC], f32, tag="lnscratch")
                nc.scalar.activation(out=lnscratch[:], in_=s32[:], func=Ln,
                                     accum_out=acc_ls[:, c:c + 1])
                s16 = pp.tile([P, O_LOC], f16, tag=f"sig16_{c}")
                nc.vector.tensor_copy(out=s16[:], in_=s32[:])
                sig16.append(s16[:])
                m16 = pp.tile([P, O_LOC], f16, tag=f"mu16_{c}")
                nc.vector.tensor_copy(out=m16[:], in_=m32[:])
                mu16.append(m16[:])
                xx16 = pp.tile([P, B], f16, tag=f"x16_{c}")
                nc.vector.tensor_copy(out=xx16[:], in_=xx32[:])
                x16.append(xx16[:])
            mu32 = [t[:] for t in mu32]
            x32 = [t[:] for t in x32]

            # ---- bias pipeline ----
            bmu = pp.tile([P, 1], f32, tag="bmu")
            nc.sync.dma_start(out=bmu[:], in_=bmu_d[:])
            brho = pp.tile([P, 1], f32, tag="brho")
            nc.sync.dma_start(out=brho[:], in_=brho_d[:])
            eb = pp.tile([P, B], f32, tag="eb")
            nc.sync.dma_start(out=eb[:], in_=epsb_d[:])

            erb = wp.tile([P, 1], f32, tag="erb")
            nc.scalar.activation(out=erb[:], in_=brho[:], func=Exp)
            bsig = pp.tile([P, 1], f32, tag="bsig")
            nc.scalar.activation(out=bsig[:], in_=erb[:], func=Ln, bias=1.0)
            lnb = wp.tile([P, 1], f32, tag="lnb")
            nc.scalar.activation(out=lnb[:], in_=bsig[:], func=Ln,
                                 accum_out=acc_bias[:, 3:4])
            wb = pp.tile([P, B], f32, tag="wb")
            nc.vector.tensor_scalar(out=wb[:], in0=eb[:], scalar1=bsig[:, 0:1],
                                    scalar2=bmu[:, 0:1], op0=mult, op1=add)
            wb2 = wp.tile([P, B], f32, tag="wb2")
            nc.vector.scalar_tensor_tensor(out=wb2[:], in0=wb[:], scalar=1.0,
                                           in1=wb[:], op0=mult, op1=mult,
                                           accum_out=acc_bias[:, 0:1])
            Eb = wp.tile([P, B], f32, tag="Eb")
            nc.scalar.activation(out=Eb[:], in_=wb2[:], func=Exp,
                                 scale=-C2, bias=c1_ap[:, 0:1])
            Lb = wp.tile([P, B], f32, tag="Lb")
            nc.scalar.activation(out=Lb[:], in_=Eb[:], func=Ln, bias=1.0,
                                 accum_out=acc_bias[:, 1:2])
            eb2 = wp.tile([P, B], f32, tag="eb2")
            nc.vector.scalar_tensor_tensor(out=eb2[:], in0=eb[:], scalar=1.0,
                                           in1=eb[:], op0=mult, op1=mult,
                                           accum_out=acc_bias[:, 2:3])

            # ---- dense mu matmul: out[o,b] = sum_i mu[o,i] x[b,i] (fp32) ----
            for c in range(NCH):
                nc.tensor.matmul(out=mv_ps[:], lhsT=mu32[c], rhs=x32[c],
                                 start=(c == 0), stop=False)

            # ---- main eps stream ----
            for t in range(NTILES):
                c, g = divmod(t, NG)
                use_pair = (t % PAIR_EVERY) < PAIR_CNT
                e16 = bigp.tile([P, BG * O_LOC], f16, tag="e16")
                nc.sync.dma_start(out=e16[:], in_=eps_d[c, g])

                e16v = e16[:].rearrange("p (b o) -> p b o", b=BG)
                sig_bc = sig16[c].unsqueeze(1).broadcast_to([P, BG, O_LOC])
                mu_bc = mu16[c].unsqueeze(1).broadcast_to([P, BG, O_LOC])

                s16 = bigp.tile([P, BG * O_LOC], f16, tag="s16")
                s16v = s16[:].rearrange("p (b o) -> p b o", b=BG)
                nc.vector.tensor_tensor(out=s16v, in0=e16v, in1=sig_bc, op=mult)

                # w2 = (s + mu)^2 with fused sum -> acc_w2[:, t]
                w2 = wp.tile([P, BG * O_LOC], f16, tag="w2")
                w2v = w2[:].rearrange("p (b o) -> p b o", b=BG)
                nc.vector._custom_dve(OP_SQADD, out=w2v, in0=s16v, in1=mu_bc,
                                      accum_out=acc_w2[:, t:t + 1])

                E = wp.tile([P, BG * O_LOC], f16, tag="E")
                nc.scalar.activation(out=E[:], in_=w2[:], func=Exp,
                                     scale=-C2, bias=c1_ap[:, 0:1])
                if use_pair:
                    Ev = E[:].rearrange("p (n two) -> p n two", two=2)
                    q = smp.tile([P, BG * O_LOC // 2], bf16, tag="q")
                    nc.vector._custom_dve(OP_PAIR, out=q[:], in0=Ev[:, :, 0],
                                          in1=Ev[:, :, 1], s0=1.0)
                    L = smp.tile([P, BG * O_LOC // 2], f16, tag="Lp")
                    nc.scalar.activation(out=L[:], in_=q[:], func=Ln,
                                         accum_out=acc_L[:, t:t + 1])
                else:
                    L = smp.tile([P, BG * O_LOC], f16, tag="L")
                    nc.scalar.activation(out=L[:], in_=E[:], func=Ln, bias=1.0,
                                         accum_out=acc_L[:, t:t + 1])

                for bl in range(BG):
                    b = g * BG + bl
                    last = (t == NTILES - 1 and bl == BG - 1)
                    nc.tensor.matmul(out=mv_ps[:, b:b + 1],
                                     lhsT=s16v[:, bl, :],
                                     rhs=x16[c][:, b:b + 1],
                                     start=False, stop=last)
                    nc.tensor.matmul(out=gram_ps[:],
                                     lhsT=e16v[:, bl, :],
                                     rhs=e16v[:, bl, :],
                                     start=(t == 0 and bl == 0), stop=last)

            # ---- finalize ----
            gmask = wp.tile([P, P], f32, tag="gmask")
            nc.vector.tensor_tensor(out=gmask[:], in0=gram_ps[:], in1=ident[:],
                                    op=mult)
            nc.vector.tensor_reduce(out=acc_e2[:], in_=gmask[:],
                                    axis=mybir.AxisListType.X,
                                    op=add)

            out_sb = wp.tile([O_LOC, B], f32, tag="out_sb")
            nc.vector.tensor_tensor(out=out_sb[:], in0=mv_ps[:], in1=wb[:], op=add)
            nc.sync.dma_start(out=out_d[:], in_=out_sb[:])

            pk = smp.tile([P, 2 * NTILES + 1 + NCH + 4], f32, tag="pk")
            nc.vector.tensor_copy(out=pk[:, 0:NTILES], in_=acc_w2[:])
            nc.vector.tensor_copy(out=pk[:, NTILES:2 * NTILES], in_=acc_L[:])
            nc.vector.tensor_copy(out=pk[:, 2 * NTILES:2 * NTILES + 1], in_=acc_e2[:])
            nc.vector.tensor_copy(out=pk[:, 2 * NTILES + 1:2 * NTILES + 1 + NCH], in_=acc_ls[:])
            nc.vector.tensor_copy(out=pk[:, 2 * NTILES + 1 + NCH:], in_=acc_bias[:])
            nc.sync.dma_start(out=par_d[:], in_=pk[:])

    nc.compile()
    _CACHE["nc"] = nc
    return nc


def _prepare_in_maps(x, weight_mu, weight_rho, bias_mu, bias_rho, eps_w, eps_b):
    x = np.asarray(x, np.float32)
    weight_mu = np.asarray(weight_mu, np.float32)
    weight_rho = np.asarray(weight_rho, np.float32)
    bias_mu = np.asarray(bias_mu, np.float32)
    bias_rho = np.asarray(bias_rho, np.float32)
    eps_w = np.asarray(eps_w, np.float32)
    eps_b = np.asarray(eps_b, np.float32)

    # x_t[c, p, b] = x[b, 128c+p]  (shared by all cores)
    x_t = np.ascontiguousarray(x.reshape(B, NCH, P).transpose(1, 2, 0))
    eps16 = eps_w.astype(np.float16)

    in_maps = []
    for r in range(NCORES):
        osh = slice(r * O_LOC, (r + 1) * O_LOC)
        # eps_t[c, g, p, bl*O_LOC + o] = eps16[g*BG+bl, osh.start+o, 128c+p]
        shard = eps16[:, osh, :]                       # (B, O_LOC, IN)
        eps_t = np.ascontiguousarray(
            shard.reshape(NG, BG, O_LOC, NCH, P).transpose(3, 0, 4, 1, 2)
            .reshape(NCH, NG, P, BG * O_LOC))
        mu_t = np.ascontiguousarray(weight_mu[osh].T.reshape(NCH, P, O_LOC))
        rho_t = np.ascontiguousarray(weight_rho[osh].T.reshape(NCH, P, O_LOC))
        in_maps.append({
            "eps_t": eps_t,
            "mu_t": mu_t,
            "rho_t": rho_t,
            "x_t": x_t,
            "eps_b_t": np.ascontiguousarray(eps_b[:, osh].T),
            "b_mu": np.ascontiguousarray(bias_mu[osh].reshape(O_LOC, 1)),
            "b_rho": np.ascontiguousarray(bias_rho[osh].reshape(O_LOC, 1)),
        })
    return in_maps


def kernel(x, weight_mu, weight_rho, bias_mu, bias_rho, eps_w, eps_b):
    global LAST_EXEC_NS
    from concourse.bass_utils import run_bass_kernel_spmd

    nc = _build()
    in_maps = _prepare_in_maps(x, weight_mu, weight_rho, bias_mu, bias_rho,
                               eps_w, eps_b)
    trace = os.environ.get("BL_TRACE", "0") == "1"
    kw = {}
    td = os.environ.get("BL_TMPDIR")
    if td:
        os.makedirs(td, exist_ok=True)
        kw["tmpdir"] = td
    res = run_bass_kernel_spmd(nc, in_maps, list(range(NCORES)), trace=trace, **kw)
    LAST_EXEC_NS = res.exec_time_ns
    _CACHE["last_results"] = res

    out = np.concatenate([res.results[r]["out_t"].T for r in range(NCORES)],
                         axis=1).astype(np.float32)

    sw2 = sL = se2 = sls = 0.0
    swb2 = sLb = seb2 = slbs = 0.0
    for r in range(NCORES):
        p = res.results[r]["partials"].astype(np.float64)
        sw2 += p[:, 0:NTILES].sum()
        sL += p[:, NTILES:2 * NTILES].sum()
        se2 += p[:, 2 * NTILES].sum()
        sls += p[:, 2 * NTILES + 1:2 * NTILES + 1 + NCH].sum()
        swb2 += p[:, 2 * NTILES + 1 + NCH + 0].sum()
        sLb += p[:, 2 * NTILES + 1 + NCH + 1].sum()
        seb2 += p[:, 2 * NTILES + 1 + NCH + 2].sum()
        slbs += p[:, 2 * NTILES + 1 + NCH + 3].sum()

    n_w = float(B) * OUT * IN
    n_b = float(B) * OUT
    log_prior = (A_CONST * (n_w + n_b) - 0.5 * (sw2 + swb2) + (sL + sLb))
    log_posterior = (-LOG_SQRT_2PI * (n_w + n_b)
                     - B * (sls + slbs) - 0.5 * (se2 + seb2))
    return out, np.float32(log_prior), np.float32(log_posterior)


# revision 12
# speedup vs baseline: 1.0948x; 1.0135x over previous
"""Bayesian linear layer (sample branch) on 8 Trainium2 NeuronCores.

Sharding: 8-way over the OUT dimension (128 output features per core).
The 256MB eps_w tensor is pre-cast to fp16 and pre-tiled on the host, so
each core streams a contiguous 16MB tile sequence at full DMA rate:

  eps_t[c, g, p, (bl o)] = fp16(eps_w[g*BG+bl, o_shard+o, 128c+p])

Per tile [128 i-part, (BG b x 128 o) free]:
    DVE : s   = sigmaT (x) eps16              (fp16 TT, 2x mode)
          w2, Sw2 = SQADD custom op: sq(s + muT) with fused accumulate
          (on ~40% of tiles) q = (1+E) pairwise products (PAIR custom op)
    ACT : E   = exp(-c2*w2 + c1)              (one table set: exp+ln)
          SL += ln(E + 1)  or  ln(q) on paired tiles
    PE  : out[o,b] += s_tile^T @ x_col        (matvec, PSUM accum)
          Se2 via eps-Gram diag: psum += e_tile^T @ e_tile
plus a dense fp32 mu-matmul on PE and a tiny bias pipeline.  Scalar
partial sums leave as per-partition vectors, reduced on the host.

Exact identities:
  log_prior elem = a + softplus(d),  a = log(.5)-LOG_SQRT_2PI - w^2/2,
                   d = -log(.002) - (125000-0.5) w^2
  ln(1+E1) + ln(1+E2) = ln((1+E1)(1+E2))
  log_posterior  = -N*LOG_SQRT_2PI - B*Sum log sigma - Sum eps^2 / 2
"""

import os
import numpy as np

B, IN, OUT = 64, 1024, 1024
NCORES = 8
O_LOC = OUT // NCORES      # 128
P = 128
NCH = IN // P              # 8 i-chunks
BG = 32                    # batches per tile
NG = B // BG               # 2 tile groups over batch
NTILES = NCH * NG          # 16
PAIR_EVERY = 6             # tiles with (t % PAIR_EVERY) < PAIR_CNT use pairing
PAIR_CNT = 1

LOG_SQRT_2PI = 0.9189385332046727
C1 = 6.214608098422191     # -log(0.002)
C2 = 124999.5              # 1/(2*0.002^2) - 1/2
A_CONST = float(np.log(0.5) - LOG_SQRT_2PI)

_CACHE = {}
LAST_EXEC_NS = None


def _patch_act_tables():
    """Force every activation onto the one table set that holds both exp
    and ln, so the kernel does a single ACT_TABLE_LOAD instead of
    thrashing between exp_and_others and natural_log per instruction."""
    import concourse.bacc as bacc_mod

    if getattr(bacc_mod, "_ant_single_act_set", False):
        return
    orig = bacc_mod.get_activation_tables

    def patched(arch):
        t = orig(arch)
        return {
            name: (fns if name == "natural_log_exp_and_others" else set())
            for name, fns in t.items()
        }

    bacc_mod.get_activation_tables = patched
    bacc_mod._ant_single_act_set = True


def _register_custom_ops():
    """Register two fused DVE micro-op programs:
       SQADD_REDUCE_ANT: out = (in0 + in1)^2 ; accum_out = sum(out)
       PAIR1P_ANT:       out = (in0 + s0) * (in1 + s0)
    Appended to dve_ops.OPS at runtime; shas computed on the fly."""
    from concourse import dve_ops as dops
    from concourse.dve_spec import Spec, Src0, Src1, Zero, C0, sq, lower
    from concourse.dve_spec import _has_src1 as has_src1
    from concourse.dve_uop import DveOpSpec
    from operator import add as _add

    if "SQADD_REDUCE_ANT" in dops._SUB_OPCODE_FOR_NAME:
        by_name = {op.name: op for op in dops.OPS}
        return by_name["SQADD_REDUCE_ANT"], by_name["PAIR1P_ANT"]

    def _ref_sqadd(in0, in1, c0, c1, c2):
        b = ((in0.astype(np.float32) + in1) ** 2).astype(np.float32)
        return b, b.reshape(b.shape[0], -1).sum(axis=-1, keepdims=True)

    def _ref_pair(in0, in1, s0, s1, imm2):
        return (in0.astype(np.float32) + s0) * (in1.astype(np.float32) + s0)

    def _register(name, spec, subdim=False):
        row = max(dops._SUB_OPCODE_FOR_NAME.values()) + 1
        assert row < 0x20
        dops._SUB_OPCODE_FOR_NAME[name] = row
        shas = {}
        for ver in ("v3", "v4"):
            s = DveOpSpec(name=name, opcode=row, uops=lower(spec, ver=ver),
                          rd1_en=has_src1(spec))
            shas[ver] = s.sha(ver)
        op = dops.DveOp(name, spec, subdim=subdim, uops_sha=shas)
        dops.OPS.append(op)
        dops.CUSTOM_DVE_SPECS[name] = spec
        return op

    sqadd = _register(
        "SQADD_REDUCE_ANT",
        Spec(body=sq(Src0 + Src1), accum=_add, accum_init=Zero,
             reference=_ref_sqadd))
    pair = _register(
        "PAIR1P_ANT",
        Spec(body=(Src0 + C0) * (Src1 + C0), reference=_ref_pair))
    return sqadd, pair


def _build():
    if "nc" in _CACHE:
        return _CACHE["nc"]
    _patch_act_tables()
    OP_SQADD, OP_PAIR = _register_custom_ops()
    import concourse.mybir as mybir
    from concourse import bacc
    from concourse.tile import TileContext
    from concourse.masks import make_identity

    f32, f16 = mybir.dt.float32, mybir.dt.float16
    bf16 = mybir.dt.bfloat16
    mult, add = mybir.AluOpType.mult, mybir.AluOpType.add
    Exp, Ln = mybir.ActivationFunctionType.Exp, mybir.ActivationFunctionType.Ln

    nc = bacc.Bacc("TRN2", target_bir_lowering=False, debug=False,
                   num_devices=NCORES)

    eps_d = nc.declare_dram_parameter("eps_t", [NCH, NG, P, BG * O_LOC], f16, isOutput=False)
    mu_d = nc.declare_dram_parameter("mu_t", [NCH, P, O_LOC], f32, isOutput=False)
    rho_d = nc.declare_dram_parameter("rho_t", [NCH, P, O_LOC], f32, isOutput=False)
    x_d = nc.declare_dram_parameter("x_t", [NCH, P, B], f32, isOutput=False)
    epsb_d = nc.declare_dram_parameter("eps_b_t", [O_LOC, B], f32, isOutput=False)
    bmu_d = nc.declare_dram_parameter("b_mu", [O_LOC, 1], f32, isOutput=False)
    brho_d = nc.declare_dram_parameter("b_rho", [O_LOC, 1], f32, isOutput=False)

    out_d = nc.declare_dram_parameter("out_t", [O_LOC, B], f32, isOutput=True)
    NPAR = 2 * NTILES + 1 + NCH + 4
    par_d = nc.declare_dram_parameter("partials", [P, NPAR], f32, isOutput=True)

    with TileContext(nc) as tc:
        with tc.tile_pool(name="persist", bufs=1) as pp, \
             tc.tile_pool(name="big", bufs=4) as bigp, \
             tc.tile_pool(name="work", bufs=3) as wp, \
             tc.tile_pool(name="small", bufs=2) as smp, \
             tc.tile_pool(name="psum", bufs=1, space="PSUM") as psp:

            c1_ap = pp.tile([P, 1], f32, tag="c1")
            nc.vector.memset(c1_ap[:], C1)
            ident = pp.tile([P, P], f32, tag="ident")
            make_identity(nc, ident[:])

            acc_w2 = pp.tile([P, NTILES], f32, tag="acc_w2")
            acc_L = pp.tile([P, NTILES], f32, tag="acc_L")
            acc_e2 = pp.tile([P, 1], f32, tag="acc_e2")
            acc_ls = pp.tile([P, NCH], f32, tag="acc_ls")
            acc_bias = pp.tile([P, 4], f32, tag="acc_bias")

            mv_ps = psp.tile([O_LOC, B], f32, tag="mv")
            gram_ps = psp.tile([P, P], f32, tag="gram")

            # ---- preamble: sigma/mu/x per i-chunk ----
            sig16 = []
            mu16 = []
            x16 = []
            mu32 = []
            x32 = []
            for c in range(NCH):
                m32 = pp.tile([P, O_LOC], f32, tag=f"mu32_{c}")
                nc.sync.dma_start(out=m32[:], in_=mu_d[c])
                mu32.append(m32)
                xx32 = pp.tile([P, B], f32, tag=f"x32_{c}")
                nc.sync.dma_start(out=xx32[:], in_=x_d[c])
                x32.append(xx32)

                rho = wp.tile([P, O_LOC], f32, tag="rho")
                nc.sync.dma_start(out=rho[:], in_=rho_d[c])
                er = wp.tile([P, O_LOC], f32, tag="er")
                nc.scalar.activation(out=er[:], in_=rho[:], func=Exp)
                s32 = wp.tile([P, O_LOC], f32, tag="s32")
                nc.scalar.activation(out=s32[:], in_=er[:], func=Ln, bias=1.0)
                lnscratch = wp.tile([P, O_LOC], f32, tag="lnscratch")
                nc.scalar.activation(out=lnscratch[:], in_=s32[:], func=Ln,
                                     accum_out=acc_ls[:, c:c + 1])
                s16 = pp.tile([P, O_LOC], f16, tag=f"sig16_{c}")
                nc.vector.tensor_copy(out=s16[:], in_=s32[:])
                sig16.append(s16[:])
                m16 = pp.tile([P, O_LOC], f16, tag=f"mu16_{c}")
                nc.vector.tensor_copy(out=m16[:], in_=m32[:])
                mu16.append(m16[:])
                xx16 = pp.tile([P, B], f16, tag=f"x16_{c}")
                nc.vector.tensor_copy(out=xx16[:], in_=xx32[:])
                x16.append(xx16[:])
            mu32 = [t[:] for t in mu32]
            x32 = [t[:] for t in x32]

            # ---- bias pipeline ----
            bmu = pp.tile([P, 1], f32, tag="bmu")
            nc.sync.dma_start(out=bmu[:], in_=bmu_d[:])
            brho = pp.tile([P, 1], f32, tag="brho")
            nc.sync.dma_start(out=brho[:], in_=brho_d[:])
            eb = pp.tile([P, B], f32, tag="eb")
            nc.sync.dma_start(out=eb[:], in_=epsb_d[:])

            erb = wp.tile([P, 1], f32, tag="erb")
            nc.scalar.activation(out=erb[:], in_=brho[:], func=Exp)
            bsig = pp.tile([P, 1], f32, tag="bsig")
            nc.scalar.activation(out=bsig[:], in_=erb[:], func=Ln, bias=1.0)
            lnb = wp.tile([P, 1], f32, tag="lnb")
            nc.scalar.activation(out=lnb[:], in_=bsig[:], func=Ln,
                                 accum_out=acc_bias[:, 3:4])
            wb = pp.tile([P, B], f32, tag="wb")
            nc.vector.tensor_scalar(out=wb[:], in0=eb[:], scalar1=bsig[:, 0:1],
                                    scalar2=bmu[:, 0:1], op0=mult, op1=add)
            wb2 = wp.tile([P, B], f32, tag="wb2")
            nc.vector.scalar_tensor_tensor(out=wb2[:], in0=wb[:], scalar=1.0,
                                           in1=wb[:], op0=mult, op1=mult,
                                           accum_out=acc_bias[:, 0:1])
            Eb = wp.tile([P, B], f32, tag="Eb")
            nc.scalar.activation(out=Eb[:], in_=wb2[:], func=Exp,
                                 scale=-C2, bias=c1_ap[:, 0:1])
            Lb = wp.tile([P, B], f32, tag="Lb")
            nc.scalar.activation(out=Lb[:], in_=Eb[:], func=Ln, bias=1.0,
                                 accum_out=acc_bias[:, 1:2])
            eb2 = wp.tile([P, B], f32, tag="eb2")
            nc.vector.scalar_tensor_tensor(out=eb2[:], in0=eb[:], scalar=1.0,
                                           in1=eb[:], op0=mult, op1=mult,
                                           accum_out=acc_bias[:, 2:3])

            # ---- dense mu matmul: out[o,b] = sum_i mu[o,i] x[b,i] (fp32) ----
            for c in range(NCH):
                nc.tensor.matmul(out=mv_ps[:], lhsT=mu32[c], rhs=x32[c],
                                 start=(c == 0), stop=False)

            # ---- main eps stream ----
            for t in range(NTILES):
                c, g = divmod(t, NG)
                use_pair = (t % PAIR_EVERY) < PAIR_CNT
                e16 = bigp.tile([P, BG * O_LOC], f16, tag="e16")
                nc.sync.dma_start(out=e16[:], in_=eps_d[c, g])

                e16v = e16[:].rearrange("p (b o) -> p b o", b=BG)
                sig_bc = sig16[c].unsqueeze(1).broadcast_to([P, BG, O_LOC])
                mu_bc = mu16[c].unsqueeze(1).broadcast_to([P, BG, O_LOC])

                s16 = bigp.tile([P, BG * O_LOC], f16, tag="s16")
                s16v = s16[:].rearrange("p (b o) -> p b o", b=BG)
                nc.vector.tensor_tensor(out=s16v, in0=e16v, in1=sig_bc, op=mult)

                # w2 = (s + mu)^2 with fused sum -> acc_w2[:, t]
                w2 = wp.tile([P, BG * O_LOC], f16, tag="w2")
                w2v = w2[:].rearrange("p (b o) -> p b o", b=BG)
                nc.vector._custom_dve(OP_SQADD, out=w2v, in0=s16v, in1=mu_bc,
                                      accum_out=acc_w2[:, t:t + 1])

                E = wp.tile([P, BG * O_LOC], f16, tag="E")
                nc.scalar.activation(out=E[:], in_=w2[:], func=Exp,
                                     scale=-C2, bias=c1_ap[:, 0:1])
                if use_pair:
                    Ev = E[:].rearrange("p (n two) -> p n two", two=2)
                    q = smp.tile([P, BG * O_LOC // 2], bf16, tag="q")
                    nc.vector._custom_dve(OP_PAIR, out=q[:], in0=Ev[:, :, 0],
                                          in1=Ev[:, :, 1], s0=1.0)
                    L = smp.tile([P, BG * O_LOC // 2], f16, tag="Lp")
                    nc.scalar.activation(out=L[:], in_=q[:], func=Ln,
                                         accum_out=acc_L[:, t:t + 1])
                else:
                    L = smp.tile([P, BG * O_LOC], f16, tag="L")
                    nc.scalar.activation(out=L[:], in_=E[:], func=Ln, bias=1.0,
                                         accum_out=acc_L[:, t:t + 1])

                for bl in range(BG):
                    b = g * BG + bl
                    last = (t == NTILES - 1 and bl == BG - 1)
                    nc.tensor.matmul(out=mv_ps[:, b:b + 1],
                                     lhsT=s16v[:, bl, :],
                                     rhs=x16[c][:, b:b + 1],
                                     start=False, stop=last)
                    nc.tensor.matmul(out=gram_ps[:],
                                     lhsT=e16v[:, bl, :],
                                     rhs=e16v[:, bl, :],
                                     start=(t == 0 and bl == 0), stop=last)

            # ---- finalize ----
            gmask = wp.tile([P, P], f32, tag="gmask")
            nc.vector.tensor_tensor(out=gmask[:], in0=gram_ps[:], in1=ident[:],
                                    op=mult)
            nc.vector.tensor_reduce(out=acc_e2[:], in_=gmask[:],
                                    axis=mybir.AxisListType.X,
                                    op=add)

            out_sb = wp.tile([O_LOC, B], f32, tag="out_sb")
            nc.vector.tensor_tensor(out=out_sb[:], in0=mv_ps[:], in1=wb[:], op=add)
            nc.sync.dma_start(out=out_d[:], in_=out_sb[:])

            pk = smp.tile([P, 2 * NTILES + 1 + NCH + 4], f32, tag="pk")
            nc.vector.tensor_copy(out=pk[:, 0:NTILES], in_=acc_w2[:])
            nc.vector.tensor_copy(out=pk[:, NTILES:2 * NTILES], in_=acc_L[:])
            nc.vector.tensor_copy(out=pk[:, 2 * NTILES:2 * NTILES + 1], in_=acc_e2[:])
            nc.vector.tensor_copy(out=pk[:, 2 * NTILES + 1:2 * NTILES + 1 + NCH], in_=acc_ls[:])
            nc.vector.tensor_copy(out=pk[:, 2 * NTILES + 1 + NCH:], in_=acc_bias[:])
            nc.sync.dma_start(out=par_d[:], in_=pk[:])

    nc.compile()
    _CACHE["nc"] = nc
    return nc


def _prepare_in_maps(x, weight_mu, weight_rho, bias_mu, bias_rho, eps_w, eps_b):
    x = np.asarray(x, np.float32)
    weight_mu = np.asarray(weight_mu, np.float32)
    weight_rho = np.asarray(weight_rho, np.float32)
    bias_mu = np.asarray(bias_mu, np.float32)
    bias_rho = np.asarray(bias_rho, np.float32)
    eps_w = np.asarray(eps_w, np.float32)
    eps_b = np.asarray(eps_b, np.float32)

    # x_t[c, p, b] = x[b, 128c+p]  (shared by all cores)
    x_t = np.ascontiguousarray(x.reshape(B, NCH, P).transpose(1, 2, 0))
    eps16 = eps_w.astype(np.float16)

    in_maps = []
    for r in range(NCORES):
        osh = slice(r * O_LOC, (r + 1) * O_LOC)
        # eps_t[c, g, p, bl*O_LOC + o] = eps16[g*BG+bl, osh.start+o, 128c+p]
        shard = eps16[:, osh, :]                       # (B, O_LOC, IN)
        eps_t = np.ascontiguousarray(
            shard.reshape(NG, BG, O_LOC, NCH, P).transpose(3, 0, 4, 1, 2)
            .reshape(NCH, NG, P, BG * O_LOC))
        mu_t = np.ascontiguousarray(weight_mu[osh].T.reshape(NCH, P, O_LOC))
        rho_t = np.ascontiguousarray(weight_rho[osh].T.reshape(NCH, P, O_LOC))
        in_maps.append({
            "eps_t": eps_t,
            "mu_t": mu_t,
            "rho_t": rho_t,
            "x_t": x_t,
            "eps_b_t": np.ascontiguousarray(eps_b[:, osh].T),
            "b_mu": np.ascontiguousarray(bias_mu[osh].reshape(O_LOC, 1)),
            "b_rho": np.ascontiguousarray(bias_rho[osh].reshape(O_LOC, 1)),
        })
    return in_maps


def kernel(x, weight_mu, weight_rho, bias_mu, bias_rho, eps_w, eps_b):
    global LAST_EXEC_NS
    from concourse.bass_utils import run_bass_kernel_spmd

    nc = _build()
    in_maps = _prepare_in_maps(x, weight_mu, weight_rho, bias_mu, bias_rho,
                               eps_w, eps_b)
    trace = os.environ.get("BL_TRACE", "0") == "1"
    kw = {}
    td = os.environ.get("BL_TMPDIR")
    if td:
        os.makedirs(td, exist_ok=True)
        kw["tmpdir"] = td
    res = run_bass_kernel_spmd(nc, in_maps, list(range(NCORES)), trace=trace, **kw)
    LAST_EXEC_NS = res.exec_time_ns
    _CACHE["last_results"] = res

    out = np.concatenate([res.results[r]["out_t"].T for r in range(NCORES)],
                         axis=1).astype(np.float32)

    sw2 = sL = se2 = sls = 0.0
    swb2 = sLb = seb2 = slbs = 0.0
    for r in range(NCORES):
        p = res.results[r]["partials"].astype(np.float64)
        sw2 += p[:, 0:NTILES].sum()
        sL += p[:, NTILES:2 * NTILES].sum()
        se2 += p[:, 2 * NTILES].sum()
        sls += p[:, 2 * NTILES + 1:2 * NTILES + 1 + NCH].sum()
        swb2 += p[:, 2 * NTILES + 1 + NCH + 0].sum()
        sLb += p[:, 2 * NTILES + 1 + NCH + 1].sum()
        seb2 += p[:, 2 * NTILES + 1 + NCH + 2].sum()
        slbs += p[:, 2 * NTILES + 1 + NCH + 3].sum()

    n_w = float(B) * OUT * IN
    n_b = float(B) * OUT
    log_prior = (A_CONST * (n_w + n_b) - 0.5 * (sw2 + swb2) + (sL + sLb))
    log_posterior = (-LOG_SQRT_2PI * (n_w + n_b)
                     - B * (sls + slbs) - 0.5 * (se2 + seb2))
    return out, np.float32(log_prior), np.float32(log_posterior)


# revision 13
# speedup vs baseline: 1.1205x; 1.0235x over previous
"""Bayesian linear layer (sample branch) on 8 Trainium2 NeuronCores.

Sharding: 8-way over the OUT dimension (128 output features per core).
The 256MB eps_w tensor is pre-cast to fp16 and pre-tiled on the host, so
each core streams a contiguous 16MB tile sequence at full DMA rate:

  eps_t[c, g, p, (bl o)] = fp16(eps_w[g*BG+bl, o_shard+o, 128c+p])

Per tile [128 i-part, (BG b x 128 o) free]:
    DVE : s   = sigmaT (x) eps16              (fp16 TT, 2x mode)
          w2, Sw2 = SQADD custom op: sq(s + muT) with fused accumulate
          (on ~40% of tiles) q = (1+E) pairwise products (PAIR custom op)
    ACT : E   = exp(-c2*w2 + c1)              (one table set: exp+ln)
          SL += ln(E + 1)  or  ln(q) on paired tiles
    PE  : out[o,b] += s_tile^T @ x_col        (matvec, PSUM accum)
          Se2 via eps-Gram diag: psum += e_tile^T @ e_tile
plus a dense fp32 mu-matmul on PE and a tiny bias pipeline.  Scalar
partial sums leave as per-partition vectors, reduced on the host.

Exact identities:
  log_prior elem = a + softplus(d),  a = log(.5)-LOG_SQRT_2PI - w^2/2,
                   d = -log(.002) - (125000-0.5) w^2
  ln(1+E1) + ln(1+E2) = ln((1+E1)(1+E2))
  log_posterior  = -N*LOG_SQRT_2PI - B*Sum log sigma - Sum eps^2 / 2
"""

import os
import numpy as np

B, IN, OUT = 64, 1024, 1024
NCORES = 8
O_LOC = OUT // NCORES      # 128
P = 128
NCH = IN // P              # 8 i-chunks
BG = 32                    # batches per tile
NG = B // BG               # 2 tile groups over batch
NTILES = NCH * NG          # 16
PAIR_EVERY = 6             # tiles with (t % PAIR_EVERY) < PAIR_CNT use pairing
PAIR_CNT = 1

LOG_SQRT_2PI = 0.9189385332046727
C1 = 6.214608098422191     # -log(0.002)
C2 = 124999.5              # 1/(2*0.002^2) - 1/2
A_CONST = float(np.log(0.5) - LOG_SQRT_2PI)

_CACHE = {}
LAST_EXEC_NS = None


def _patch_act_tables():
    """Force every activation onto the one table set that holds both exp
    and ln, so the kernel does a single ACT_TABLE_LOAD instead of
    thrashing between exp_and_others and natural_log per instruction."""
    import concourse.bacc as bacc_mod

    if getattr(bacc_mod, "_ant_single_act_set", False):
        return
    orig = bacc_mod.get_activation_tables

    def patched(arch):
        t = orig(arch)
        return {
            name: (fns if name == "natural_log_exp_and_others" else set())
            for name, fns in t.items()
        }

    bacc_mod.get_activation_tables = patched
    bacc_mod._ant_single_act_set = True


def _register_custom_ops():
    """Register two fused DVE micro-op programs:
       SQADD_REDUCE_ANT: out = (in0 + in1)^2 ; accum_out = sum(out)
       PAIR1P_ANT:       out = (in0 + s0) * (in1 + s0)
    Appended to dve_ops.OPS at runtime; shas computed on the fly."""
    from concourse import dve_ops as dops
    from concourse.dve_spec import Spec, Src0, Src1, Zero, C0, sq, lower
    from concourse.dve_spec import _has_src1 as has_src1
    from concourse.dve_uop import DveOpSpec
    from operator import add as _add

    if "SQADD_REDUCE_ANT" in dops._SUB_OPCODE_FOR_NAME:
        by_name = {op.name: op for op in dops.OPS}
        return by_name["SQADD_REDUCE_ANT"], by_name["PAIR1P_ANT"]

    def _ref_sqadd(in0, in1, c0, c1, c2):
        b = ((in0.astype(np.float32) + in1) ** 2).astype(np.float32)
        return b, b.reshape(b.shape[0], -1).sum(axis=-1, keepdims=True)

    def _ref_pair(in0, in1, s0, s1, imm2):
        return (in0.astype(np.float32) + s0) * (in1.astype(np.float32) + s0)

    def _register(name, spec, subdim=False):
        row = max(dops._SUB_OPCODE_FOR_NAME.values()) + 1
        assert row < 0x20
        dops._SUB_OPCODE_FOR_NAME[name] = row
        shas = {}
        for ver in ("v3", "v4"):
            s = DveOpSpec(name=name, opcode=row, uops=lower(spec, ver=ver),
                          rd1_en=has_src1(spec))
            shas[ver] = s.sha(ver)
        op = dops.DveOp(name, spec, subdim=subdim, uops_sha=shas)
        dops.OPS.append(op)
        dops.CUSTOM_DVE_SPECS[name] = spec
        return op

    sqadd = _register(
        "SQADD_REDUCE_ANT",
        Spec(body=sq(Src0 + Src1), accum=_add, accum_init=Zero,
             reference=_ref_sqadd))
    pair = _register(
        "PAIR1P_ANT",
        Spec(body=(Src0 + C0) * (Src1 + C0), reference=_ref_pair))
    return sqadd, pair


def _build():
    if "nc" in _CACHE:
        return _CACHE["nc"]
    _patch_act_tables()
    OP_SQADD, OP_PAIR = _register_custom_ops()
    import concourse.mybir as mybir
    from concourse import bacc
    from concourse.tile import TileContext
    from concourse.masks import make_identity

    f32, f16 = mybir.dt.float32, mybir.dt.float16
    bf16 = mybir.dt.bfloat16
    mult, add = mybir.AluOpType.mult, mybir.AluOpType.add
    Exp, Ln = mybir.ActivationFunctionType.Exp, mybir.ActivationFunctionType.Ln

    nc = bacc.Bacc("TRN2", target_bir_lowering=False, debug=False,
                   num_devices=NCORES)

    eps_d = nc.declare_dram_parameter("eps_t", [NCH, NG, P, BG * O_LOC], f16, isOutput=False)
    mu_d = nc.declare_dram_parameter("mu_t", [NCH, P, O_LOC], f32, isOutput=False)
    rho_d = nc.declare_dram_parameter("rho_t", [NCH, P, O_LOC], f32, isOutput=False)
    x_d = nc.declare_dram_parameter("x_t", [NCH, P, B], f32, isOutput=False)
    epsb_d = nc.declare_dram_parameter("eps_b_t", [O_LOC, B], f32, isOutput=False)
    bmu_d = nc.declare_dram_parameter("b_mu", [O_LOC, 1], f32, isOutput=False)
    brho_d = nc.declare_dram_parameter("b_rho", [O_LOC, 1], f32, isOutput=False)

    out_d = nc.declare_dram_parameter("out_t", [O_LOC, B], f32, isOutput=True)
    NPAR = 2 * NTILES + 1 + NCH + 4
    par_d = nc.declare_dram_parameter("partials", [P, NPAR], f32, isOutput=True)

    with TileContext(nc) as tc:
        with tc.tile_pool(name="persist", bufs=1) as pp, \
             tc.tile_pool(name="big", bufs=4) as bigp, \
             tc.tile_pool(name="work", bufs=3) as wp, \
             tc.tile_pool(name="small", bufs=2) as smp, \
             tc.tile_pool(name="psum", bufs=1, space="PSUM") as psp:

            # issue the first eps-tile DMAs before anything else so the
            # SDMA engines start streaming immediately (the 24 preamble
            # DMA issues otherwise delay tile 0 by ~15us on the SP queue)
            pre_e16 = {}
            for t in range(4):
                c, g = divmod(t, NG)
                e = bigp.tile([P, BG * O_LOC], f16, tag="e16")
                nc.sync.dma_start(out=e[:], in_=eps_d[c, g])
                pre_e16[t] = e

            c1_ap = pp.tile([P, 1], f32, tag="c1")
            nc.vector.memset(c1_ap[:], C1)
            ident = pp.tile([P, P], f32, tag="ident")
            make_identity(nc, ident[:])

            acc_w2 = pp.tile([P, NTILES], f32, tag="acc_w2")
            acc_L = pp.tile([P, NTILES], f32, tag="acc_L")
            acc_e2 = pp.tile([P, 1], f32, tag="acc_e2")
            acc_ls = pp.tile([P, NCH], f32, tag="acc_ls")
            acc_bias = pp.tile([P, 4], f32, tag="acc_bias")

            mv_ps = psp.tile([O_LOC, B], f32, tag="mv")
            gram_ps = psp.tile([P, P], f32, tag="gram")

            # ---- preamble: sigma/mu/x per i-chunk ----
            sig16 = []
            mu16 = []
            x16 = []
            mu32 = []
            x32 = []
            for c in range(NCH):
                m32 = pp.tile([P, O_LOC], f32, tag=f"mu32_{c}")
                nc.sync.dma_start(out=m32[:], in_=mu_d[c])
                mu32.append(m32)
                xx32 = pp.tile([P, B], f32, tag=f"x32_{c}")
                nc.sync.dma_start(out=xx32[:], in_=x_d[c])
                x32.append(xx32)

                rho = wp.tile([P, O_LOC], f32, tag="rho")
                nc.sync.dma_start(out=rho[:], in_=rho_d[c])
                er = wp.tile([P, O_LOC], f32, tag="er")
                nc.scalar.activation(out=er[:], in_=rho[:], func=Exp)
                s32 = wp.tile([P, O_LOC], f32, tag="s32")
                nc.scalar.activation(out=s32[:], in_=er[:], func=Ln, bias=1.0)
                lnscratch = wp.tile([P, O_LOC], f32, tag="lnscratch")
                nc.scalar.activation(out=lnscratch[:], in_=s32[:], func=Ln,
                                     accum_out=acc_ls[:, c:c + 1])
                s16 = pp.tile([P, O_LOC], f16, tag=f"sig16_{c}")
                nc.vector.tensor_copy(out=s16[:], in_=s32[:])
                sig16.append(s16[:])
                m16 = pp.tile([P, O_LOC], f16, tag=f"mu16_{c}")
                nc.vector.tensor_copy(out=m16[:], in_=m32[:])
                mu16.append(m16[:])
                xx16 = pp.tile([P, B], f16, tag=f"x16_{c}")
                nc.vector.tensor_copy(out=xx16[:], in_=xx32[:])
                x16.append(xx16[:])
            mu32 = [t[:] for t in mu32]
            x32 = [t[:] for t in x32]

            # ---- bias pipeline ----
            bmu = pp.tile([P, 1], f32, tag="bmu")
            nc.sync.dma_start(out=bmu[:], in_=bmu_d[:])
            brho = pp.tile([P, 1], f32, tag="brho")
            nc.sync.dma_start(out=brho[:], in_=brho_d[:])
            eb = pp.tile([P, B], f32, tag="eb")
            nc.sync.dma_start(out=eb[:], in_=epsb_d[:])

            erb = wp.tile([P, 1], f32, tag="erb")
            nc.scalar.activation(out=erb[:], in_=brho[:], func=Exp)
            bsig = pp.tile([P, 1], f32, tag="bsig")
            nc.scalar.activation(out=bsig[:], in_=erb[:], func=Ln, bias=1.0)
            lnb = wp.tile([P, 1], f32, tag="lnb")
            nc.scalar.activation(out=lnb[:], in_=bsig[:], func=Ln,
                                 accum_out=acc_bias[:, 3:4])
            wb = pp.tile([P, B], f32, tag="wb")
            nc.vector.tensor_scalar(out=wb[:], in0=eb[:], scalar1=bsig[:, 0:1],
                                    scalar2=bmu[:, 0:1], op0=mult, op1=add)
            wb2 = wp.tile([P, B], f32, tag="wb2")
            nc.vector.scalar_tensor_tensor(out=wb2[:], in0=wb[:], scalar=1.0,
                                           in1=wb[:], op0=mult, op1=mult,
                                           accum_out=acc_bias[:, 0:1])
            Eb = wp.tile([P, B], f32, tag="Eb")
            nc.scalar.activation(out=Eb[:], in_=wb2[:], func=Exp,
                                 scale=-C2, bias=c1_ap[:, 0:1])
            Lb = wp.tile([P, B], f32, tag="Lb")
            nc.scalar.activation(out=Lb[:], in_=Eb[:], func=Ln, bias=1.0,
                                 accum_out=acc_bias[:, 1:2])
            eb2 = wp.tile([P, B], f32, tag="eb2")
            nc.vector.scalar_tensor_tensor(out=eb2[:], in0=eb[:], scalar=1.0,
                                           in1=eb[:], op0=mult, op1=mult,
                                           accum_out=acc_bias[:, 2:3])

            # ---- dense mu matmul: out[o,b] = sum_i mu[o,i] x[b,i] (fp32) ----
            for c in range(NCH):
                nc.tensor.matmul(out=mv_ps[:], lhsT=mu32[c], rhs=x32[c],
                                 start=(c == 0), stop=False)

            # ---- main eps stream ----
            for t in range(NTILES):
                c, g = divmod(t, NG)
                use_pair = (t % PAIR_EVERY) < PAIR_CNT
                if t in pre_e16:
                    e16 = pre_e16.pop(t)
                else:
                    e16 = bigp.tile([P, BG * O_LOC], f16, tag="e16")
                    nc.sync.dma_start(out=e16[:], in_=eps_d[c, g])

                e16v = e16[:].rearrange("p (b o) -> p b o", b=BG)
                sig_bc = sig16[c].unsqueeze(1).broadcast_to([P, BG, O_LOC])
                mu_bc = mu16[c].unsqueeze(1).broadcast_to([P, BG, O_LOC])

                s16 = bigp.tile([P, BG * O_LOC], f16, tag="s16")
                s16v = s16[:].rearrange("p (b o) -> p b o", b=BG)
                nc.vector.tensor_tensor(out=s16v, in0=e16v, in1=sig_bc, op=mult)

                # w2 = (s + mu)^2 with fused sum -> acc_w2[:, t]
                w2 = wp.tile([P, BG * O_LOC], f16, tag="w2")
                w2v = w2[:].rearrange("p (b o) -> p b o", b=BG)
                nc.vector._custom_dve(OP_SQADD, out=w2v, in0=s16v, in1=mu_bc,
                                      accum_out=acc_w2[:, t:t + 1])

                E = wp.tile([P, BG * O_LOC], f16, tag="E")
                nc.scalar.activation(out=E[:], in_=w2[:], func=Exp,
                                     scale=-C2, bias=c1_ap[:, 0:1])
                if use_pair:
                    Ev = E[:].rearrange("p (n two) -> p n two", two=2)
                    q = smp.tile([P, BG * O_LOC // 2], bf16, tag="q")
                    nc.vector._custom_dve(OP_PAIR, out=q[:], in0=Ev[:, :, 0],
                                          in1=Ev[:, :, 1], s0=1.0)
                    L = smp.tile([P, BG * O_LOC // 2], f16, tag="Lp")
                    nc.scalar.activation(out=L[:], in_=q[:], func=Ln,
                                         accum_out=acc_L[:, t:t + 1])
                else:
                    L = smp.tile([P, BG * O_LOC], f16, tag="L")
                    nc.scalar.activation(out=L[:], in_=E[:], func=Ln, bias=1.0,
                                         accum_out=acc_L[:, t:t + 1])

                for bl in range(BG):
                    b = g * BG + bl
                    last = (t == NTILES - 1 and bl == BG - 1)
                    nc.tensor.matmul(out=mv_ps[:, b:b + 1],
                                     lhsT=s16v[:, bl, :],
                                     rhs=x16[c][:, b:b + 1],
                                     start=False, stop=last)
                    nc.tensor.matmul(out=gram_ps[:],
                                     lhsT=e16v[:, bl, :],
                                     rhs=e16v[:, bl, :],
                                     start=(t == 0 and bl == 0), stop=last)

            # ---- finalize ----
            gmask = wp.tile([P, P], f32, tag="gmask")
            nc.vector.tensor_tensor(out=gmask[:], in0=gram_ps[:], in1=ident[:],
                                    op=mult)
            nc.vector.tensor_reduce(out=acc_e2[:], in_=gmask[:],
                                    axis=mybir.AxisListType.X,
                                    op=add)

            out_sb = wp.tile([O_LOC, B], f32, tag="out_sb")
            nc.vector.tensor_tensor(out=out_sb[:], in0=mv_ps[:], in1=wb[:], op=add)
            nc.sync.dma_start(out=out_d[:], in_=out_sb[:])

            pk = smp.tile([P, 2 * NTILES + 1 + NCH + 4], f32, tag="pk")
            nc.vector.tensor_copy(out=pk[:, 0:NTILES], in_=acc_w2[:])
            nc.vector.tensor_copy(out=pk[:, NTILES:2 * NTILES], in_=acc_L[:])
            nc.vector.tensor_copy(out=pk[:, 2 * NTILES:2 * NTILES + 1], in_=acc_e2[:])
            nc.vector.tensor_copy(out=pk[:, 2 * NTILES + 1:2 * NTILES + 1 + NCH], in_=acc_ls[:])
            nc.vector.tensor_copy(out=pk[:, 2 * NTILES + 1 + NCH:], in_=acc_bias[:])
            nc.sync.dma_start(out=par_d[:], in_=pk[:])

    nc.compile()
    _CACHE["nc"] = nc
    return nc


def _prepare_in_maps(x, weight_mu, weight_rho, bias_mu, bias_rho, eps_w, eps_b):
    x = np.asarray(x, np.float32)
    weight_mu = np.asarray(weight_mu, np.float32)
    weight_rho = np.asarray(weight_rho, np.float32)
    bias_mu = np.asarray(bias_mu, np.float32)
    bias_rho = np.asarray(bias_rho, np.float32)
    eps_w = np.asarray(eps_w, np.float32)
    eps_b = np.asarray(eps_b, np.float32)

    # x_t[c, p, b] = x[b, 128c+p]  (shared by all cores)
    x_t = np.ascontiguousarray(x.reshape(B, NCH, P).transpose(1, 2, 0))
    eps16 = eps_w.astype(np.float16)

    in_maps = []
    for r in range(NCORES):
        osh = slice(r * O_LOC, (r + 1) * O_LOC)
        # eps_t[c, g, p, bl*O_LOC + o] = eps16[g*BG+bl, osh.start+o, 128c+p]
        shard = eps16[:, osh, :]                       # (B, O_LOC, IN)
        eps_t = np.ascontiguousarray(
            shard.reshape(NG, BG, O_LOC, NCH, P).transpose(3, 0, 4, 1, 2)
            .reshape(NCH, NG, P, BG * O_LOC))
        mu_t = np.ascontiguousarray(weight_mu[osh].T.reshape(NCH, P, O_LOC))
        rho_t = np.ascontiguousarray(weight_rho[osh].T.reshape(NCH, P, O_LOC))
        in_maps.append({
            "eps_t": eps_t,
            "mu_t": mu_t,
            "rho_t": rho_t,
            "x_t": x_t,
            "eps_b_t": np.ascontiguousarray(eps_b[:, osh].T),
            "b_mu": np.ascontiguousarray(bias_mu[osh].reshape(O_LOC, 1)),
            "b_rho": np.ascontiguousarray(bias_rho[osh].reshape(O_LOC, 1)),
        })
    return in_maps


def kernel(x, weight_mu, weight_rho, bias_mu, bias_rho, eps_w, eps_b):
    global LAST_EXEC_NS
    from concourse.bass_utils import run_bass_kernel_spmd

    nc = _build()
    in_maps = _prepare_in_maps(x, weight_mu, weight_rho, bias_mu, bias_rho,
                               eps_w, eps_b)
    trace = os.environ.get("BL_TRACE", "0") == "1"
    kw = {}
    td = os.environ.get("BL_TMPDIR")
    if td:
        os.makedirs(td, exist_ok=True)
        kw["tmpdir"] = td
    res = run_bass_kernel_spmd(nc, in_maps, list(range(NCORES)), trace=trace, **kw)
    LAST_EXEC_NS = res.exec_time_ns
    _CACHE["last_results"] = res

    out = np.concatenate([res.results[r]["out_t"].T for r in range(NCORES)],
                         axis=1).astype(np.float32)

    sw2 = sL = se2 = sls = 0.0
    swb2 = sLb = seb2 = slbs = 0.0
    for r in range(NCORES):
        p = res.results[r]["partials"].astype(np.float64)
        sw2 += p[:, 0:NTILES].sum()
        sL += p[:, NTILES:2 * NTILES].sum()
        se2 += p[:, 2 * NTILES].sum()
        sls += p[:, 2 * NTILES + 1:2 * NTILES + 1 + NCH].sum()
        swb2 += p[:, 2 * NTILES + 1 + NCH + 0].sum()
        sLb += p[:, 2 * NTILES + 1 + NCH + 1].sum()
        seb2 += p[:, 2 * NTILES + 1 + NCH + 2].sum()
        slbs += p[:, 2 * NTILES + 1 + NCH + 3].sum()

    n_w = float(B) * OUT * IN
    n_b = float(B) * OUT
    log_prior = (A_CONST * (n_w + n_b) - 0.5 * (sw2 + swb2) + (sL + sLb))
    log_posterior = (-LOG_SQRT_2PI * (n_w + n_b)
                     - B * (sls + slbs) - 0.5 * (se2 + seb2))
    return out, np.float32(log_prior), np.float32(log_posterior)


# revision 16
# speedup vs baseline: 1.1768x; 1.0503x over previous
"""Bayesian linear layer (sample branch) on 8 Trainium2 NeuronCores.

Sharding: 8-way over the OUT dimension (128 output features per core).
The 256MB eps_w tensor is pre-cast to fp16 and pre-tiled on the host, so
each core streams a contiguous 16MB tile sequence at full DMA rate:

  eps_t[c, g, p, (bl o)] = fp16(eps_w[g*BG+bl, o_shard+o, 128c+p])

Per tile [128 i-part, (BG b x 128 o) free]:
    DVE : s   = sigmaT (x) eps16              (fp16 TT, 2x mode)
          w2, Sw2 = SQADD custom op: sq(s + muT) with fused accumulate
          (on ~40% of tiles) q = (1+E) pairwise products (PAIR custom op)
    ACT : E   = exp(-c2*w2 + c1)              (one table set: exp+ln)
          SL += ln(E + 1)  or  ln(q) on paired tiles
    PE  : out[o,b] += s_tile^T @ x_col        (matvec, PSUM accum)
          Se2 via eps-Gram diag: psum += e_tile^T @ e_tile
plus a dense fp32 mu-matmul on PE and a tiny bias pipeline.  Scalar
partial sums leave as per-partition vectors, reduced on the host.

Exact identities:
  log_prior elem = a + softplus(d),  a = log(.5)-LOG_SQRT_2PI - w^2/2,
                   d = -log(.002) - (125000-0.5) w^2
  ln(1+E1) + ln(1+E2) = ln((1+E1)(1+E2))
  log_posterior  = -N*LOG_SQRT_2PI - B*Sum log sigma - Sum eps^2 / 2
"""

import os
import numpy as np

B, IN, OUT = 64, 1024, 1024
NCORES = 8
O_LOC = OUT // NCORES      # 128
P = 128
NCH = IN // P              # 8 i-chunks
BG = 32                    # batches per tile
NG = B // BG               # 2 tile groups over batch
NTILES = NCH * NG          # 16
PAIR_EVERY = 6             # tiles with (t % PAIR_EVERY) < PAIR_CNT use pairing
PAIR_CNT = 1

LOG_SQRT_2PI = 0.9189385332046727
C1 = 6.214608098422191     # -log(0.002)
C2 = 124999.5              # 1/(2*0.002^2) - 1/2
A_CONST = float(np.log(0.5) - LOG_SQRT_2PI)

_CACHE = {}
LAST_EXEC_NS = None


def _patch_act_tables():
    """Force every activation onto the one table set that holds both exp
    and ln, so the kernel does a single ACT_TABLE_LOAD instead of
    thrashing between exp_and_others and natural_log per instruction."""
    import concourse.bacc as bacc_mod

    if getattr(bacc_mod, "_ant_single_act_set", False):
        return
    orig = bacc_mod.get_activation_tables

    def patched(arch):
        t = orig(arch)
        return {
            name: (fns if name == "natural_log_exp_and_others" else set())
            for name, fns in t.items()
        }

    bacc_mod.get_activation_tables = patched
    bacc_mod._ant_single_act_set = True


def _register_custom_ops():
    """Register two fused DVE micro-op programs:
       SQADD_REDUCE_ANT: out = (in0 + in1)^2 ; accum_out = sum(out)
       PAIR1P_ANT:       out = (in0 + s0) * (in1 + s0)
    Appended to dve_ops.OPS at runtime; shas computed on the fly."""
    from concourse import dve_ops as dops
    from concourse.dve_spec import Spec, Src0, Src1, Zero, C0, sq, lower
    from concourse.dve_spec import _has_src1 as has_src1
    from concourse.dve_uop import DveOpSpec
    from operator import add as _add

    if "SQADD_REDUCE_ANT" in dops._SUB_OPCODE_FOR_NAME:
        by_name = {op.name: op for op in dops.OPS}
        return by_name["SQADD_REDUCE_ANT"], by_name["PAIR1P_ANT"]

    def _ref_sqadd(in0, in1, c0, c1, c2):
        b = ((in0.astype(np.float32) + in1) ** 2).astype(np.float32)
        return b, b.reshape(b.shape[0], -1).sum(axis=-1, keepdims=True)

    def _ref_pair(in0, in1, s0, s1, imm2):
        return (in0.astype(np.float32) + s0) * (in1.astype(np.float32) + s0)

    def _register(name, spec, subdim=False):
        row = max(dops._SUB_OPCODE_FOR_NAME.values()) + 1
        assert row < 0x20
        dops._SUB_OPCODE_FOR_NAME[name] = row
        shas = {}
        for ver in ("v3", "v4"):
            s = DveOpSpec(name=name, opcode=row, uops=lower(spec, ver=ver),
                          rd1_en=has_src1(spec))
            shas[ver] = s.sha(ver)
        op = dops.DveOp(name, spec, subdim=subdim, uops_sha=shas)
        dops.OPS.append(op)
        dops.CUSTOM_DVE_SPECS[name] = spec
        return op

    sqadd = _register(
        "SQADD_REDUCE_ANT",
        Spec(body=sq(Src0 + Src1), accum=_add, accum_init=Zero,
             reference=_ref_sqadd))
    pair = _register(
        "PAIR1P_ANT",
        Spec(body=(Src0 + C0) * (Src1 + C0), reference=_ref_pair))
    return sqadd, pair


def _build():
    if "nc" in _CACHE:
        return _CACHE["nc"]
    _patch_act_tables()
    OP_SQADD, OP_PAIR = _register_custom_ops()
    import concourse.mybir as mybir
    from concourse import bacc
    from concourse.tile import TileContext
    from concourse.masks import make_identity

    f32, f16 = mybir.dt.float32, mybir.dt.float16
    bf16 = mybir.dt.bfloat16
    mult, add = mybir.AluOpType.mult, mybir.AluOpType.add
    Exp, Ln = mybir.ActivationFunctionType.Exp, mybir.ActivationFunctionType.Ln

    nc = bacc.Bacc("TRN2", target_bir_lowering=False, debug=False,
                   num_devices=NCORES)

    eps_d = nc.declare_dram_parameter("eps_t", [NCH, NG, P, BG * O_LOC], f16, isOutput=False)
    mu_d = nc.declare_dram_parameter("mu_t", [NCH, P, O_LOC], f32, isOutput=False)
    rho_d = nc.declare_dram_parameter("rho_t", [NCH, P, O_LOC], f32, isOutput=False)
    x_d = nc.declare_dram_parameter("x_t", [NCH, P, B], f32, isOutput=False)
    epsb_d = nc.declare_dram_parameter("eps_b_t", [O_LOC, B], f32, isOutput=False)
    bmu_d = nc.declare_dram_parameter("b_mu", [O_LOC, 1], f32, isOutput=False)
    brho_d = nc.declare_dram_parameter("b_rho", [O_LOC, 1], f32, isOutput=False)

    out_d = nc.declare_dram_parameter("out_t", [O_LOC, B], f32, isOutput=True)
    NPAR = 2 * NTILES + 1 + NCH + 4
    par_d = nc.declare_dram_parameter("partials", [P, NPAR], f32, isOutput=True)

    with TileContext(nc) as tc:
        with tc.tile_pool(name="persist", bufs=1) as pp, \
             tc.tile_pool(name="big", bufs=4) as bigp, \
             tc.tile_pool(name="work", bufs=3) as wp, \
             tc.tile_pool(name="small", bufs=2) as smp, \
             tc.tile_pool(name="psum", bufs=1, space="PSUM") as psp:

            # DMA issue order matters on the SP queue: the tiny rho tiles
            # first (they gate the sigma chain on ACT), then the first
            # eps tiles (so SDMA streams immediately), then the rest.
            rho_t = []
            for c in range(NCH):
                r = pp.tile([P, O_LOC], f32, tag=f"rho_{c}")
                nc.sync.dma_start(out=r[:], in_=rho_d[c])
                rho_t.append(r)
            pre_e16 = {}
            for t in range(4):
                c, g = divmod(t, NG)
                e = bigp.tile([P, BG * O_LOC], f16, tag="e16")
                nc.sync.dma_start(out=e[:], in_=eps_d[c, g])
                pre_e16[t] = e

            c1_ap = pp.tile([P, 1], f32, tag="c1")
            nc.vector.memset(c1_ap[:], C1)
            ident = pp.tile([P, P], f32, tag="ident")
            make_identity(nc, ident[:])

            acc_w2 = pp.tile([P, NTILES], f32, tag="acc_w2")
            acc_L = pp.tile([P, NTILES], f32, tag="acc_L")
            acc_e2 = pp.tile([P, 1], f32, tag="acc_e2")
            acc_ls = pp.tile([P, NCH], f32, tag="acc_ls")
            acc_bias = pp.tile([P, 4], f32, tag="acc_bias")

            mv_ps = psp.tile([O_LOC, B], f32, tag="mv")
            gram_ps = psp.tile([P, P], f32, tag="gram")

            # ---- preamble: sigma/mu/x per i-chunk ----
            sig16 = []
            mu16 = []
            x16 = []
            mu32 = []
            x32 = []
            for c in range(NCH):
                m32 = pp.tile([P, O_LOC], f32, tag=f"mu32_{c}")
                nc.sync.dma_start(out=m32[:], in_=mu_d[c])
                mu32.append(m32)
                xx32 = pp.tile([P, B], f32, tag=f"x32_{c}")
                nc.sync.dma_start(out=xx32[:], in_=x_d[c])
                x32.append(xx32)

                er = wp.tile([P, O_LOC], f32, tag="er")
                nc.scalar.activation(out=er[:], in_=rho_t[c][:], func=Exp)
                s32 = wp.tile([P, O_LOC], f32, tag="s32")
                nc.scalar.activation(out=s32[:], in_=er[:], func=Ln, bias=1.0)
                lnscratch = wp.tile([P, O_LOC], f32, tag="lnscratch")
                nc.scalar.activation(out=lnscratch[:], in_=s32[:], func=Ln,
                                     accum_out=acc_ls[:, c:c + 1])
                s16 = pp.tile([P, O_LOC], f16, tag=f"sig16_{c}")
                nc.vector.tensor_copy(out=s16[:], in_=s32[:])
                sig16.append(s16[:])
                m16 = pp.tile([P, O_LOC], f16, tag=f"mu16_{c}")
                nc.vector.tensor_copy(out=m16[:], in_=m32[:])
                mu16.append(m16[:])
                xx16 = pp.tile([P, B], f16, tag=f"x16_{c}")
                nc.vector.tensor_copy(out=xx16[:], in_=xx32[:])
                x16.append(xx16[:])
            mu32 = [t[:] for t in mu32]
            x32 = [t[:] for t in x32]

            # ---- bias pipeline ----
            bmu = pp.tile([P, 1], f32, tag="bmu")
            nc.sync.dma_start(out=bmu[:], in_=bmu_d[:])
            brho = pp.tile([P, 1], f32, tag="brho")
            nc.sync.dma_start(out=brho[:], in_=brho_d[:])
            eb = pp.tile([P, B], f32, tag="eb")
            nc.sync.dma_start(out=eb[:], in_=epsb_d[:])

            erb = wp.tile([P, 1], f32, tag="erb")
            nc.scalar.activation(out=erb[:], in_=brho[:], func=Exp)
            bsig = pp.tile([P, 1], f32, tag="bsig")
            nc.scalar.activation(out=bsig[:], in_=erb[:], func=Ln, bias=1.0)
            lnb = wp.tile([P, 1], f32, tag="lnb")
            nc.scalar.activation(out=lnb[:], in_=bsig[:], func=Ln,
                                 accum_out=acc_bias[:, 3:4])
            wb = pp.tile([P, B], f32, tag="wb")
            nc.vector.tensor_scalar(out=wb[:], in0=eb[:], scalar1=bsig[:, 0:1],
                                    scalar2=bmu[:, 0:1], op0=mult, op1=add)
            wb2 = wp.tile([P, B], f32, tag="wb2")
            nc.vector.scalar_tensor_tensor(out=wb2[:], in0=wb[:], scalar=1.0,
                                           in1=wb[:], op0=mult, op1=mult,
                                           accum_out=acc_bias[:, 0:1])
            Eb = wp.tile([P, B], f32, tag="Eb")
            nc.scalar.activation(out=Eb[:], in_=wb2[:], func=Exp,
                                 scale=-C2, bias=c1_ap[:, 0:1])
            Lb = wp.tile([P, B], f32, tag="Lb")
            nc.scalar.activation(out=Lb[:], in_=Eb[:], func=Ln, bias=1.0,
                                 accum_out=acc_bias[:, 1:2])
            eb2 = wp.tile([P, B], f32, tag="eb2")
            nc.vector.scalar_tensor_tensor(out=eb2[:], in0=eb[:], scalar=1.0,
                                           in1=eb[:], op0=mult, op1=mult,
                                           accum_out=acc_bias[:, 2:3])

            # ---- dense mu matmul: out[o,b] = sum_i mu[o,i] x[b,i] (fp32) ----
            for c in range(NCH):
                nc.tensor.matmul(out=mv_ps[:], lhsT=mu32[c], rhs=x32[c],
                                 start=(c == 0), stop=False)

            # ---- main eps stream ----
            for t in range(NTILES):
                c, g = divmod(t, NG)
                use_pair = ((t % PAIR_EVERY) < PAIR_CNT) or (t >= NTILES - 3)
                if t in pre_e16:
                    e16 = pre_e16.pop(t)
                else:
                    e16 = bigp.tile([P, BG * O_LOC], f16, tag="e16")
                    nc.sync.dma_start(out=e16[:], in_=eps_d[c, g])

                e16v = e16[:].rearrange("p (b o) -> p b o", b=BG)
                sig_bc = sig16[c].unsqueeze(1).broadcast_to([P, BG, O_LOC])
                mu_bc = mu16[c].unsqueeze(1).broadcast_to([P, BG, O_LOC])

                s16 = bigp.tile([P, BG * O_LOC], f16, tag="s16")
                s16v = s16[:].rearrange("p (b o) -> p b o", b=BG)
                nc.vector.tensor_tensor(out=s16v, in0=e16v, in1=sig_bc, op=mult)

                # w2 = (s + mu)^2 with fused sum -> acc_w2[:, t]
                w2 = wp.tile([P, BG * O_LOC], f16, tag="w2")
                w2v = w2[:].rearrange("p (b o) -> p b o", b=BG)
                nc.vector._custom_dve(OP_SQADD, out=w2v, in0=s16v, in1=mu_bc,
                                      accum_out=acc_w2[:, t:t + 1])

                E = wp.tile([P, BG * O_LOC], f16, tag="E")
                nc.scalar.activation(out=E[:], in_=w2[:], func=Exp,
                                     scale=-C2, bias=c1_ap[:, 0:1])
                if use_pair:
                    Ev = E[:].rearrange("p (n two) -> p n two", two=2)
                    q = smp.tile([P, BG * O_LOC // 2], bf16, tag="q")
                    nc.vector._custom_dve(OP_PAIR, out=q[:], in0=Ev[:, :, 0],
                                          in1=Ev[:, :, 1], s0=1.0)
                    L = smp.tile([P, BG * O_LOC // 2], f16, tag="Lp")
                    nc.scalar.activation(out=L[:], in_=q[:], func=Ln,
                                         accum_out=acc_L[:, t:t + 1])
                else:
                    L = smp.tile([P, BG * O_LOC], f16, tag="L")
                    nc.scalar.activation(out=L[:], in_=E[:], func=Ln, bias=1.0,
                                         accum_out=acc_L[:, t:t + 1])

                for bl in range(BG):
                    b = g * BG + bl
                    last = (t == NTILES - 1 and bl == BG - 1)
                    nc.tensor.matmul(out=mv_ps[:, b:b + 1],
                                     lhsT=s16v[:, bl, :],
                                     rhs=x16[c][:, b:b + 1],
                                     start=False, stop=last)
                    nc.tensor.matmul(out=gram_ps[:],
                                     lhsT=e16v[:, bl, :],
                                     rhs=e16v[:, bl, :],
                                     start=(t == 0 and bl == 0), stop=last)

            # ---- finalize ----
            gmask = wp.tile([P, P], f32, tag="gmask")
            nc.vector.tensor_tensor(out=gmask[:], in0=gram_ps[:], in1=ident[:],
                                    op=mult)
            nc.vector.tensor_reduce(out=acc_e2[:], in_=gmask[:],
                                    axis=mybir.AxisListType.X,
                                    op=add)

            out_sb = wp.tile([O_LOC, B], f32, tag="out_sb")
            nc.vector.tensor_tensor(out=out_sb[:], in0=mv_ps[:], in1=wb[:], op=add)
            nc.sync.dma_start(out=out_d[:], in_=out_sb[:])

            pk = smp.tile([P, 2 * NTILES + 1 + NCH + 4], f32, tag="pk")
            nc.vector.tensor_copy(out=pk[:, 0:NTILES], in_=acc_w2[:])
            nc.vector.tensor_copy(out=pk[:, NTILES:2 * NTILES], in_=acc_L[:])
            nc.vector.tensor_copy(out=pk[:, 2 * NTILES:2 * NTILES + 1], in_=acc_e2[:])
            nc.vector.tensor_copy(out=pk[:, 2 * NTILES + 1:2 * NTILES + 1 + NCH], in_=acc_ls[:])
            nc.vector.tensor_copy(out=pk[:, 2 * NTILES + 1 + NCH:], in_=acc_bias[:])
            nc.sync.dma_start(out=par_d[:], in_=pk[:])

    nc.compile()
    _CACHE["nc"] = nc
    return nc


def _prepare_in_maps(x, weight_mu, weight_rho, bias_mu, bias_rho, eps_w, eps_b):
    x = np.asarray(x, np.float32)
    weight_mu = np.asarray(weight_mu, np.float32)
    weight_rho = np.asarray(weight_rho, np.float32)
    bias_mu = np.asarray(bias_mu, np.float32)
    bias_rho = np.asarray(bias_rho, np.float32)
    eps_w = np.asarray(eps_w, np.float32)
    eps_b = np.asarray(eps_b, np.float32)

    # x_t[c, p, b] = x[b, 128c+p]  (shared by all cores)
    x_t = np.ascontiguousarray(x.reshape(B, NCH, P).transpose(1, 2, 0))
    eps16 = eps_w.astype(np.float16)

    in_maps = []
    for r in range(NCORES):
        osh = slice(r * O_LOC, (r + 1) * O_LOC)
        # eps_t[c, g, p, bl*O_LOC + o] = eps16[g*BG+bl, osh.start+o, 128c+p]
        shard = eps16[:, osh, :]                       # (B, O_LOC, IN)
        eps_t = np.ascontiguousarray(
            shard.reshape(NG, BG, O_LOC, NCH, P).transpose(3, 0, 4, 1, 2)
            .reshape(NCH, NG, P, BG * O_LOC))
        mu_t = np.ascontiguousarray(weight_mu[osh].T.reshape(NCH, P, O_LOC))
        rho_t = np.ascontiguousarray(weight_rho[osh].T.reshape(NCH, P, O_LOC))
        in_maps.append({
            "eps_t": eps_t,
            "mu_t": mu_t,
            "rho_t": rho_t,
            "x_t": x_t,
            "eps_b_t": np.ascontiguousarray(eps_b[:, osh].T),
            "b_mu": np.ascontiguousarray(bias_mu[osh].reshape(O_LOC, 1)),
            "b_rho": np.ascontiguousarray(bias_rho[osh].reshape(O_LOC, 1)),
        })
    return in_maps


def kernel(x, weight_mu, weight_rho, bias_mu, bias_rho, eps_w, eps_b):
    global LAST_EXEC_NS
    from concourse.bass_utils import run_bass_kernel_spmd

    nc = _build()
    in_maps = _prepare_in_maps(x, weight_mu, weight_rho, bias_mu, bias_rho,
                               eps_w, eps_b)
    trace = os.environ.get("BL_TRACE", "0") == "1"
    kw = {}
    td = os.environ.get("BL_TMPDIR")
    if td:
        os.makedirs(td, exist_ok=True)
        kw["tmpdir"] = td
    res = run_bass_kernel_spmd(nc, in_maps, list(range(NCORES)), trace=trace, **kw)
    LAST_EXEC_NS = res.exec_time_ns
    _CACHE["last_results"] = res

    out = np.concatenate([res.results[r]["out_t"].T for r in range(NCORES)],
                         axis=1).astype(np.float32)

    sw2 = sL = se2 = sls = 0.0
    swb2 = sLb = seb2 = slbs = 0.0
    for r in range(NCORES):
        p = res.results[r]["partials"].astype(np.float64)
        sw2 += p[:, 0:NTILES].sum()
        sL += p[:, NTILES:2 * NTILES].sum()
        se2 += p[:, 2 * NTILES].sum()
        sls += p[:, 2 * NTILES + 1:2 * NTILES + 1 + NCH].sum()
        swb2 += p[:, 2 * NTILES + 1 + NCH + 0].sum()
        sLb += p[:, 2 * NTILES + 1 + NCH + 1].sum()
        seb2 += p[:, 2 * NTILES + 1 + NCH + 2].sum()
        slbs += p[:, 2 * NTILES + 1 + NCH + 3].sum()

    n_w = float(B) * OUT * IN
    n_b = float(B) * OUT
    log_prior = (A_CONST * (n_w + n_b) - 0.5 * (sw2 + swb2) + (sL + sLb))
    log_posterior = (-LOG_SQRT_2PI * (n_w + n_b)
                     - B * (sls + slbs) - 0.5 * (se2 + seb2))
    return out, np.float32(log_prior), np.float32(log_posterior)
